# revision 49
# baseline (speedup 1.0000x reference)
"""DeepIRT forward as a Bass/Tile kernel on 8 Trainium2 NeuronCores.

Sharding: pure data parallelism over students (B=4096 -> 8 cores).
Students are globally sorted by qid_len (descending) and dealt to cores so
that every core has an IDENTICAL length profile (dummy students pad the
profile); this lets one SPMD program use a compile-time ragged schedule for
the LSTM (step t only touches the first n_t sorted columns).

Per-core program layout (P students, P % 16 == 0):
  - students indexed g in [0,P); duo D = g//4 holds 4 students (a = g%4)
  - attention (per "batch" of 4 duos = 16 students):
      qid+stu rows DMA'd -> PE transpose -> qidT [64d, 51] per student
      kemb rows via dma_gather from knE -> cast bf16 (stage-2 lhsT)
                                        -> PE transpose -> kembT (stage-1 lhsT)
      stage1: scoresT[k,t] (+ mastery preact col) via 4 quadrant matmuls/duo
              + one bias-row matmul adding -1e9 to invalid k rows
      softmax: exp (masked by bias), denominators via ones-matmul,
               reciprocal, broadcast-back via matmul
      stage2: [bvecT | mastvec | avec] via 4 quadrant matmuls/duo
  - theta/a DNNs: shared-weight matmuls over all students at once
  - LSTM: 50 steps, students split lo(even g)/hi(odd g) column groups,
          ragged active prefix per step, gates on PSUM, ACT sigmoids/tanh
  - head: b = 4*tanh((h@L_Wo+bo)/2), p = sigmoid(4 * a4 * (theta - b4))

Outputs [1, P] per core are gathered and inverse-permuted on the host.
"""

import sys
import hashlib

import numpy as np
import ml_dtypes

for _p in ("/opt/trn_rl_repo",):
    if _p not in sys.path:
        sys.path.insert(0, _p)

B, T, K, D, H, HL, S, KN = 4096, 50, 32, 64, 256, 128, 100000, 1000
N_CORES = 8
NEG = -1.0e9

_state = {}

BF16 = ml_dtypes.bfloat16


# ---------------------------------------------------------------- host prep

def _host_prep(inputs):
    """Sort/deal/pad students; build per-core input arrays + schedule."""
    lens = np.asarray(inputs["qid_len"]).astype(np.int64)          # [B]
    counts = np.bincount(lens, minlength=T + 1)                    # index 0..50
    m = -(-counts // N_CORES)                                      # ceil
    m[0] = 0
    P0 = int(m[1:].sum())
    P = ((P0 + 15) // 16) * 16
    m[1] += P - P0

    # per-core identical length profile, descending
    profile = np.repeat(np.arange(T, 0, -1), m[T:0:-1])            # [P]
    assert profile.shape[0] == P

    students = -np.ones((N_CORES, P), np.int64)
    ptr = 0
    for l in range(T, 0, -1):
        idxs = np.where(lens == l)[0]
        for c in range(N_CORES):
            take = idxs[c::N_CORES]
            assert take.shape[0] <= m[l]
            students[c, ptr:ptr + take.shape[0]] = take
        ptr += m[l]
    assert ptr == P

    n_t = np.array([(profile > t).sum() for t in range(T)], np.int64)
    nlo = [int(x) for x in (n_t + 1) // 2]
    nhi = [int(x) for x in n_t // 2]

    ND, NB = P // 4, P // 16

    qidemb = np.asarray(inputs["qidemb"], np.float32)
    stuE = np.asarray(inputs["stuE"], np.float32)
    uididx = np.asarray(inputs["uididx"])
    kcodeidx = np.asarray(inputs["kcodeidx"])
    kcode_len = np.asarray(inputs["kcode_len"]).astype(np.int64)

    # weights (replicated)
    wts = {
        "wi": np.asarray(inputs["L_Wi"], np.float32).astype(BF16),          # [64,512]
        "wh": np.asarray(inputs["L_Wh"], np.float32).astype(BF16),          # [128,512]
        "tw1": np.asarray(inputs["T_W1"], np.float32).astype(BF16),         # [64,256]
        "aw1": np.asarray(inputs["A_W1"], np.float32).astype(BF16),
        "tw2": np.asarray(inputs["T_W2"], np.float32)[:, 0].reshape(2, 128).T.copy().astype(BF16),
        "aw2": np.asarray(inputs["A_W2"], np.float32)[:, 0].reshape(2, 128).T.copy().astype(BF16),
        "lwo": np.asarray(inputs["L_Wo"], np.float32).reshape(128, 1).astype(BF16),
        "lb": np.asarray(inputs["L_b"], np.float32).reshape(4, 128).T.copy(),
        "tb1": np.asarray(inputs["T_b1"], np.float32).reshape(2, 128).T.copy(),
        "ab1": np.asarray(inputs["A_b1"], np.float32).reshape(2, 128).T.copy(),
        "scal": np.array([[float(np.asarray(inputs["T_b2"]).reshape(-1)[0]),
                           float(np.asarray(inputs["A_b2"]).reshape(-1)[0]),
                           0.5 * float(np.asarray(inputs["L_bo"]).reshape(-1)[0])]],
                         np.float32),
        "kne": np.asarray(inputs["knE"], np.float32),                       # [1000,64]
    }
    # constant pattern tiles
    sumpat = np.zeros((128, 4), BF16)
    for a in range(4):
        sumpat[32 * a:32 * (a + 1), a] = 1
    bc_ev = np.zeros((4, 128), BF16)
    bc_ev[0, 0:64] = 1
    bc_ev[1, 64:128] = 1
    bc_od = np.zeros((4, 128), BF16)
    bc_od[2, 0:64] = 1
    bc_od[3, 64:128] = 1
    blk4 = np.zeros((4, 204), BF16)
    for j in range(4):
        blk4[j, 51 * j:51 * (j + 1)] = 1
    consts = {"sumpat": sumpat, "bc_ev": bc_ev, "bc_od": bc_od, "blk4": blk4}

    in_maps = []
    for c in range(N_CORES):
        sel = students[c]
        safe = np.where(sel >= 0, sel, 0)

        q = qidemb[safe]                                           # [P,50,64]
        st = stuE[uididx[safe]]                                    # [P,64]
        qid_plus = np.concatenate([q, st[:, None, :]], axis=1).astype(BF16)

        kc = kcodeidx[safe].astype(np.int16)                       # [P,32]
        kidx = np.zeros((NB, 16, 32), np.int16)
        i = np.arange(512)
        kcb = kc.reshape(NB, 16, 32)
        kidx[:, i % 16, i // 16] = kcb[:, i // 32, i % 32]

        kl = kcode_len[safe].reshape(ND, 4)                        # [ND,4]
        kk = np.arange(K)
        kmf3 = (kk[None, None, :] < kl[:, :, None])                # [ND,4,32]
        kmf = kmf3.transpose(1, 2, 0).reshape(128, ND).astype(np.float32)
        # brow4[j, G, m] = bias of duo 4G+j at out1 row m (0 valid / -1e9 invalid)
        brow4 = np.where(kmf3, 0.0, NEG).reshape(NB, 4, 128).transpose(1, 0, 2) \
            .reshape(4, NB * 128).copy().astype(BF16)

        im = {"qid": qid_plus, "kidx": kidx, "kmf": kmf, "brow": brow4}
        im.update(wts)
        im.update(consts)
        in_maps.append(im)

    meta = {"P": P, "nlo": nlo, "nhi": nhi, "students": students}
    return in_maps, meta


# ---------------------------------------------------------------- program

def _build_program(P, nlo, nhi, phases=("attn", "dnn", "lstm", "head")):
    import os as _os
    if _os.environ.get("KPHASES"):
        phases = tuple(_os.environ["KPHASES"].split(","))
    ATT = int(_os.environ.get("KATT", "9"))
    ATT2 = int(_os.environ.get("KATT2", "9"))
    AMASK = int(_os.environ.get("KAMASK", "15"))
    import concourse.bacc as bacc
    import concourse.bass as bass
    import concourse.tile as tile
    from concourse import mybir
    from concourse.masks import make_identity
    from concourse.library_config import mlp
    from concourse.tile import add_dep_helper as add_dep
    from contextlib import ExitStack

    dt = mybir.dt
    AF = mybir.ActivationFunctionType
    ND, NB, NP2 = P // 4, P // 16, P // 2

    nc = bacc.Bacc("TRN2", target_bir_lowering=False, debug=False,
                   enable_asserts=False)

    qid = nc.dram_tensor("qid", [P, 51, 64], dt.bfloat16, kind="ExternalInput")
    kidx = nc.dram_tensor("kidx", [NB, 16, 32], dt.int16, kind="ExternalInput")
    kmf_d = nc.dram_tensor("kmf", [128, ND], dt.float32, kind="ExternalInput")
    brow_d = nc.dram_tensor("brow", [4, NB * 128], dt.bfloat16, kind="ExternalInput")
    kne = nc.dram_tensor("kne", [KN, D], dt.float32, kind="ExternalInput")
    wi_d = nc.dram_tensor("wi", [64, 512], dt.bfloat16, kind="ExternalInput")
    wh_d = nc.dram_tensor("wh", [128, 512], dt.bfloat16, kind="ExternalInput")
    tw1_d = nc.dram_tensor("tw1", [64, 256], dt.bfloat16, kind="ExternalInput")
    aw1_d = nc.dram_tensor("aw1", [64, 256], dt.bfloat16, kind="ExternalInput")
    tw2_d = nc.dram_tensor("tw2", [128, 2], dt.bfloat16, kind="ExternalInput")
    aw2_d = nc.dram_tensor("aw2", [128, 2], dt.bfloat16, kind="ExternalInput")
    lwo_d = nc.dram_tensor("lwo", [128, 1], dt.bfloat16, kind="ExternalInput")
    lb_d = nc.dram_tensor("lb", [128, 4], dt.float32, kind="ExternalInput")
    tb1_d = nc.dram_tensor("tb1", [128, 2], dt.float32, kind="ExternalInput")
    ab1_d = nc.dram_tensor("ab1", [128, 2], dt.float32, kind="ExternalInput")
    scal_d = nc.dram_tensor("scal", [1, 3], dt.float32, kind="ExternalInput")
    sumpat_d = nc.dram_tensor("sumpat", [128, 4], dt.bfloat16, kind="ExternalInput")
    bcev_d = nc.dram_tensor("bc_ev", [4, 128], dt.bfloat16, kind="ExternalInput")
    bcod_d = nc.dram_tensor("bc_od", [4, 128], dt.bfloat16, kind="ExternalInput")
    blk4_d = nc.dram_tensor("blk4", [4, 204], dt.bfloat16, kind="ExternalInput")
    out_d = nc.dram_tensor("out", [1, P], dt.float32, kind="ExternalOutput")

    with tile.TileContext(nc) as tc, ExitStack() as ctx:
        const = ctx.enter_context(tc.tile_pool(name="const", bufs=1))
        state = ctx.enter_context(tc.tile_pool(name="state", bufs=1))

        nc.gpsimd.load_library(mlp)

        ident = const.tile([128, 128], dt.bfloat16)
        make_identity(nc, ident[:])

        def load(pool, shape, dty, dram, dma2=False):
            t = pool.tile(shape, dty, tag=f"c_{dram.name}", name=f"c_{dram.name}")
            if dma2:  # duplicate 64-row weight into both partition halves
                nc.sync.dma_start(t[0:64, :], dram.ap())
                nc.sync.dma_start(t[64:128, :], dram.ap())
            else:
                nc.sync.dma_start(t[:], dram.ap())
            return t

        kmf_t = load(const, [128, ND], dt.float32, kmf_d)
        brow_t = load(const, [4, NB * 128], dt.bfloat16, brow_d)
        wi_t = load(const, [128, 512], dt.bfloat16, wi_d, dma2=True)
        wh_t = load(const, [128, 512], dt.bfloat16, wh_d)
        tw1_t = load(const, [128, 256], dt.bfloat16, tw1_d, dma2=True)
        aw1_t = load(const, [128, 256], dt.bfloat16, aw1_d, dma2=True)
        tw2_t = load(const, [128, 2], dt.bfloat16, tw2_d)
        aw2_t = load(const, [128, 2], dt.bfloat16, aw2_d)
        lwo_t = load(const, [128, 1], dt.bfloat16, lwo_d)
        lb_t = load(const, [128, 4], dt.float32, lb_d)
        tb1_t = load(const, [128, 2], dt.float32, tb1_d)
        ab1_t = load(const, [128, 2], dt.float32, ab1_d)
        scal_t = load(const, [1, 3], dt.float32, scal_d)
        sumpat_t = load(const, [128, 4], dt.bfloat16, sumpat_d)
        bcev_t = load(const, [4, 128], dt.bfloat16, bcev_d)
        bcod_t = load(const, [4, 128], dt.bfloat16, bcod_d)
        blk4_t = load(const, [4, 204], dt.bfloat16, blk4_d)

        # gather index tile: [16, n/16] block replicated into all 8
        # partition groups (each GPSIMD Q7 core reads its own group)
        kidx_t = const.tile([128, NB * 32], dt.int16)
        for rep in range(8):
            nc.sync.dma_start(
                kidx_t[16 * rep:16 * (rep + 1), :].rearrange(
                    "p (n k) -> p n k", n=NB),
                kidx.ap().rearrange("n p k -> p n k"))

        # persistent tensors
        bvec = state.tile([64, T * P], dt.bfloat16)      # [d, t*P + g]
        mastav = state.tile([64, 2 * P], dt.bfloat16)    # [d, 2*g + c]
        h_t = state.tile([128, P], dt.bfloat16)          # [HL, g]
        c_t = state.tile([128, P], dt.float32)
        theta_t = state.tile([1, P], dt.float32)
        a4_t = state.tile([1, P], dt.float32)
        b4_t = state.tile([1, P], dt.float32)
        res_t = state.tile([1, P], dt.float32)
        nc.vector.memset(h_t[:], 0.0)
        nc.vector.memset(c_t[:], 0.0)

        # ---------------- attention ----------------
        with ExitStack() as atx:
          if "attn" in phases:
            qin = atx.enter_context(tc.tile_pool(name="qin", bufs=3))
            sbA = atx.enter_context(tc.tile_pool(name="sbA", bufs=3))
            ps_q = atx.enter_context(tc.tile_pool(name="ps_q", bufs=1, space="PSUM"))
            ps_k = atx.enter_context(tc.tile_pool(name="ps_k", bufs=1, space="PSUM"))
            ps_1 = atx.enter_context(tc.tile_pool(name="ps_1", bufs=1, space="PSUM"))
            ps_2x = [atx.enter_context(tc.tile_pool(name=f"ps2x{i}", bufs=1, space="PSUM"))
                     for i in range(4)]
            ps_b = atx.enter_context(tc.tile_pool(name="ps_b", bufs=1, space="PSUM"))

            for G in range(NB):
                # qid rows for 16 students -> [102, 512] (8 pairs wide)
                qtile = qin.tile([102, 512], dt.bfloat16, tag="qtile")
                for s in range(16 if ATT >= 1 else 0):
                    nc.sync.dma_start(
                        qtile[51 * (s % 2):51 * (s % 2) + 51,
                              64 * (s // 2):64 * (s // 2) + 64],
                        qid.ap()[16 * G + s])

                # kemb gather: 512 rows of knE -> [128, 4, 64] f32
                gath = sbA.tile([128, 256], dt.float32, tag="gath")
                if ATT < 2:
                    nc.vector.memset(gath[:], 0.0)
                if ATT >= 2:
                    nc.gpsimd.dma_gather(
                        gath[:].rearrange("p (b e) -> p b e", b=4),
                        kne.ap(), kidx_t[:, 32 * G:32 * (G + 1)], 512, 512, 64)
                kc16 = sbA.tile([128, 256], dt.bfloat16, tag="kc16")
                if ATT >= 3:
                    nc.vector.tensor_copy(kc16[:], gath[:])
                else:
                    nc.vector.memset(kc16[:], 0.0)
                if ATT < 4:
                    continue

                # transposes; first per PSUM bank is start=True (marks the
                # whole bank pending-zero), the rest overwrite pending bytes
                qT_ps = ps_q.tile([64, 1024], dt.bfloat16, tag="qtps")
                first = None
                for p8 in range(8):
                    mm = nc.tensor.matmul(qT_ps[:, 102 * p8:102 * (p8 + 1)],
                                          qtile[0:102, 64 * p8:64 * (p8 + 1)],
                                          ident[0:102, 0:102],
                                          is_transpose=True,
                                          start=(p8 == 0), stop=(p8 == 7),
                                          skip_group_check=True)
                    if first is None:
                        first = mm
                    else:
                        add_dep(mm.ins, first.ins, reason="bank first-writer")
                kT_ps = ps_k.tile([64, 1024], dt.bfloat16, tag="ktps")
                first = None
                for dd in range(4):
                    mm = nc.tensor.matmul(kT_ps[:, 128 * dd:128 * (dd + 1)],
                                          kc16[0:128, 64 * dd:64 * (dd + 1)],
                                          ident[0:128, 0:128],
                                          is_transpose=True,
                                          start=(dd == 0), stop=(dd == 3),
                                          skip_group_check=True)
                    if first is None:
                        first = mm
                    else:
                        add_dep(mm.ins, first.ins, reason="bank first-writer")
                qT = sbA.tile([64, 816], dt.bfloat16, tag="qT")
                nc.vector.tensor_copy(qT[:], qT_ps[:, 0:816])
                kT = sbA.tile([64, 512], dt.bfloat16, tag="kT")
                nc.vector.tensor_copy(kT[:], kT_ps[:, 0:512])

                # stage 1: out1[32a+k, 51*dd+t] = scoresT (+ mastery col 50)
                # first writer: bias matmul filling the whole bank with the
                # -1e9 invalid-k bias (start=True), then 16 quadrant matmuls
                # accumulate the actual scores.
                if ATT < 5:
                    continue
                out1 = ps_1.tile([128, 512], dt.float32, tag="out1")
                bmm = nc.tensor.matmul(
                    out1[:, 0:204], brow_t[:, 128 * G:128 * (G + 1)], blk4_t[:],
                    start=True, stop=False, skip_group_check=True)
                for dd in range(4):
                    for a in range(4):
                        pr = 2 * dd + a // 2
                        rhs = qT[:, 102 * pr + 51 * (a % 2):102 * pr + 51 * (a % 2) + 51]
                        mm = nc.tensor.matmul(
                            out1[32 * a:32 * (a + 1), 51 * dd:51 * (dd + 1)],
                            kT[:, 128 * dd + 32 * a:128 * dd + 32 * (a + 1)],
                            rhs, start=False, stop=(dd == 3 and a == 3),
                            tile_position=(0, 32 * a), skip_group_check=True)
                        add_dep(mm.ins, bmm.ins, reason="bias first-writer")

                # softmax pieces
                if ATT < 6:
                    continue
                o1v = out1[:, 0:204].rearrange("p (d c) -> p d c", d=4)
                expw = sbA.tile([128, 208], dt.bfloat16, tag="expw")
                ewv = expw[:].rearrange("p (d c) -> p d c", d=4)
                nc.scalar.activation(ewv[:, :, 0:50], o1v[:, :, 0:50],
                                     AF.Exp, scale=0.15)
                mast = sbA.tile([128, 4], dt.float32, tag="mast")
                nc.scalar.activation(mast[:], o1v[:, :, 50:51].rearrange("p a o -> p (a o)"),
                                     AF.Sigmoid, scale=0.2)
                nc.vector.tensor_mul(ewv[:, :, 50:51].rearrange("p a o -> p (a o)"),
                                     mast[:], kmf_t[:, 4 * G:4 * G + 4])
                nc.vector.tensor_copy(ewv[:, :, 51:52].rearrange("p a o -> p (a o)"),
                                      kmf_t[:, 4 * G:4 * G + 4])

                # denominators
                if ATT < 7:
                    continue
                # denominators go into spare columns of the out1 bank
                # (rows 0-3); out1 data already consumed by exp/sigmoid
                dps = out1[0:4, 208:416]
                nc.tensor.matmul(dps, sumpat_t[:], expw[:],
                                 start=True, stop=True,
                                 skip_group_check=True)
                rden = sbA.tile([4, 208], dt.bfloat16, tag="rden")
                nc.vector.memset(rden[:], 1.0)
                with nc.allow_low_precision(reason="bf16 softmax denominators"):
                    nc.vector.reciprocal(
                        rden[:].rearrange("p (d c) -> p d c", d=4)[:, :, 0:50],
                        dps.rearrange("p (d c) -> p d c", d=4)[:, :, 0:50])
                bc = ps_b.tile([128, 512], dt.float32, tag="bc")
                mev = nc.tensor.matmul(bc[:, 0:208], bcev_t[:], rden[:],
                                       start=True, stop=False,
                                       skip_group_check=True)
                mod = nc.tensor.matmul(bc[:, 208:416], bcod_t[:], rden[:],
                                       start=False, stop=True,
                                       skip_group_check=True)
                add_dep(mod.ins, mev.ins, reason="bank first-writer")

                # stage 2: [bvecT | mastvec | avec]; first writer per
                # partition half is start=True (duo 0, a=0/a=1)
                if ATT < 8:
                    continue
                # one PSUM bank per student quadrant a (concurrent PE
                # row-tiles must not share a bank); all outputs on rows 0-63
                out2_a = [ps_2x[a].tile([64, 512], dt.float32,
                                        tag=f"o2{a}", name=f"o2{a}")
                          for a in range(4)]
                firsts = [None] * 4
                for dd in range(4):
                    for a in range(4):
                        if not (AMASK >> a) & 1:
                            continue
                        mm = nc.tensor.matmul(
                            out2_a[a][:, 52 * dd:52 * (dd + 1)],
                            kc16[32 * a:32 * (a + 1), 64 * dd:64 * (dd + 1)],
                            expw[32 * a:32 * (a + 1), 52 * dd:52 * (dd + 1)],
                            start=(firsts[a] is None), stop=True,
                            tile_position=(32 * a, 0),
                            skip_group_check=True)
                        if firsts[a] is None:
                            firsts[a] = mm
                        else:
                            add_dep(mm.ins, firsts[a].ins,
                                    reason="bank first-writer")

                # normalize bvec and write into bvec tile
                # bvec col = t*NP2 + (8G + 2*dd + par), partition-half = g%2
                # (bc must come via SBUF: DVE can't read 2 PSUM operands)
                if ATT2 < 2:
                    continue
                # bc rows: student a at partitions 64*(a%2) of col half a//2
                bc_sb = sbA.tile([128, 416], dt.bfloat16, tag="bc_sb")
                nc.vector.tensor_copy(bc_sb[:], bc[:, 0:416])
                for a in range(4):
                    r = slice(64 * (a % 2), 64 * (a % 2) + 64)
                    src = out2_a[a][:, 0:208].rearrange(
                        "p (d c) -> p d c", d=4)[:, :, 0:50]
                    bcv = bc_sb[r, 208 * (a // 2):208 * (a // 2) + 208].rearrange(
                        "p (d c) -> p d c", d=4)[:, :, 0:50]
                    dstf = bvec[0:64, :].rearrange(
                        "p (t q four) -> p q four t", q=P // 4, four=4)
                    dd_dst = dstf[:, 4 * G:4 * G + 4, a, :]
                    nc.vector.tensor_mul(dd_dst, src, bcv)

                # mastvec/avec extraction: mastav col = 2*g + c
                if ATT2 < 3:
                    continue
                mavv = mastav[:].rearrange("p (gb dd a c) -> p gb dd a c",
                                           dd=4, a=4, c=2)
                for a in range(4):
                    nc.vector.tensor_copy(
                        mavv[:, G, :, a, :],
                        out2_a[a][:, 0:208].rearrange(
                            "p (d c) -> p d c", d=4)[:, :, 50:52])

        # ---------------- theta / a DNNs ----------------
        with ExitStack() as dtx:
          if "dnn" in phases:
            sbD = dtx.enter_context(tc.tile_pool(name="sbD", bufs=2))
            ps_h = dtx.enter_context(tc.tile_pool(name="ps_h", bufs=2, space="PSUM"))
            ps_o = dtx.enter_context(tc.tile_pool(name="ps_o", bufs=2, space="PSUM"))

            mav = mastav[:].rearrange("p (s c) -> p s c", c=2)
            NCH = NP2  # column chunk (<= 512)
            for net, (w1, b1, w2, sc) in enumerate(
                    [(tw1_t, tb1_t, tw2_t, 0), (aw1_t, ab1_t, aw2_t, 1)]):
                dst_t = theta_t if net == 0 else a4_t
                for ch in range(2):
                    rhs = mav[:, NCH * ch:NCH * (ch + 1), net]     # [64, NCH]
                    ops = ps_o.tile([1, 512], dt.float32, tag="ops")
                    omm0 = None
                    for b in range(2):
                        hps = ps_h.tile([128, 512], dt.float32, tag="hps")
                        nc.tensor.matmul(hps[:, 0:NCH],
                                         w1[0:64, 128 * b:128 * (b + 1)], rhs,
                                         start=True, stop=True,
                                         tile_position=(0, 0))
                        t1b = sbD.tile([128, NCH], dt.bfloat16, tag="t1b")
                        nc.scalar.activation(t1b[:], hps[:, 0:NCH], AF.Tanh,
                                             bias=b1[:, b:b + 1])
                        omm = nc.tensor.matmul(ops[:, 0:NCH], w2[:, b:b + 1], t1b[:],
                                               start=(b == 0), stop=(b == 1),
                                               skip_group_check=True)
                        if b == 0:
                            omm0 = omm
                        else:
                            add_dep(omm.ins, omm0.ins,
                                    reason="accum first-writer")
                    nc.scalar.activation(dst_t[:, NCH * ch:NCH * (ch + 1)],
                                         ops[:, 0:NCH],
                                         AF.Identity, bias=scal_t[:, sc:sc + 1])
            # a4 = tanh(|a_pre| / 2)
            nc.scalar.activation(a4_t[:], a4_t[:], AF.Abs)
            nc.scalar.activation(a4_t[:], a4_t[:], AF.Tanh, scale=0.5)

        # ---------------- LSTM ----------------
        with ExitStack() as ltx:
          if "lstm" in phases:
            ps_g = ltx.enter_context(tc.tile_pool(name="ps_g", bufs=1, space="PSUM"))
            sbL = ltx.enter_context(tc.tile_pool(name="sbL", bufs=2))
            gps = []
            for b in range(4):
                gtile = ps_g.tile([128, 512], dt.float32,
                                  tag=f"gg{b}", name=f"gg{b}")
                gps.append(gtile)
            n_t = [a + b for a, b in zip(nlo, nhi)]
            for t in range(T):
                n = n_t[t]
                if n == 0:
                    continue
                chunks = [(0, min(n, 512))]
                if n > 512:
                    chunks.append((512, n))
                for c0, c1 in chunks:
                    w = c1 - c0
                    # Wi first: no h dependency, so the PE pre-accumulates the
                    # input projection while the previous step's gate math runs
                    for b in range(4):
                        gp = gps[b]
                        mmi = nc.tensor.matmul(gp[:, 0:w],
                                               wi_t[0:64, 128 * b:128 * (b + 1)],
                                               bvec[:, t * P + c0:t * P + c1],
                                               start=True, stop=False,
                                               tile_position=(0, 0),
                                               skip_group_check=True)
                        mmh = nc.tensor.matmul(gp[:, 0:w],
                                               wh_t[:, 128 * b:128 * (b + 1)],
                                               h_t[:, c0:c1],
                                               start=False, stop=True,
                                               skip_group_check=True)
                        add_dep(mmh.ins, mmi.ins, reason="accum first-writer")
                    si = sbL.tile([128, 512], dt.float32, tag="si")
                    sf = sbL.tile([128, 512], dt.float32, tag="sf")
                    tg = sbL.tile([128, 512], dt.float32, tag="tg")
                    so = sbL.tile([128, 512], dt.float32, tag="so")
                    nc.scalar.activation(si[:, 0:w], gps[0][:, 0:w],
                                         AF.Sigmoid, bias=lb_t[:, 0:1])
                    nc.scalar.activation(sf[:, 0:w], gps[1][:, 0:w],
                                         AF.Sigmoid, bias=lb_t[:, 1:2])
                    nc.scalar.activation(tg[:, 0:w], gps[2][:, 0:w],
                                         AF.Tanh, bias=lb_t[:, 2:3])
                    nc.scalar.activation(so[:, 0:w], gps[3][:, 0:w],
                                         AF.Sigmoid, bias=lb_t[:, 3:4])
                    t1 = sbL.tile([128, 512], dt.float32, tag="t1")
                    nc.vector.tensor_mul(t1[:, 0:w], si[:, 0:w], tg[:, 0:w])
                    nc.vector.tensor_mul(c_t[:, c0:c1], c_t[:, c0:c1],
                                         sf[:, 0:w])
                    nc.vector.tensor_add(c_t[:, c0:c1], c_t[:, c0:c1],
                                         t1[:, 0:w])
                    tc2 = sbL.tile([128, 512], dt.float32, tag="tc2")
                    nc.scalar.activation(tc2[:, 0:w], c_t[:, c0:c1], AF.Tanh)
                    nc.vector.tensor_mul(h_t[:, c0:c1], so[:, 0:w],
                                         tc2[:, 0:w])

        # ---------------- head + combine ----------------
        with ExitStack() as htx:
          if "head" in phases:
            ps_r = htx.enter_context(tc.tile_pool(name="ps_r", bufs=2, space="PSUM"))
            sbH = htx.enter_context(tc.tile_pool(name="sbH", bufs=2))
            for ch in range(2):
                bps = ps_r.tile([1, 512], dt.float32, tag="bps")
                nc.tensor.matmul(bps[:, 0:NP2], lwo_t[:],
                                 h_t[:, NP2 * ch:NP2 * (ch + 1)],
                                 start=True, stop=True)
                nc.scalar.activation(b4_t[:, NP2 * ch:NP2 * (ch + 1)],
                                     bps[:, 0:NP2], AF.Tanh,
                                     scale=0.5, bias=scal_t[:, 2:3])
            d1 = sbH.tile([1, P], dt.float32, tag="d1")
            # p = sigmoid(a*(t-b)) with a = 4*a4, b = 4*b4
            #   = sigmoid(4 * a4 * (theta - 4*b4))
            nc.vector.scalar_tensor_tensor(d1[:], b4_t[:], -4.0, theta_t[:],
                                           mybir.AluOpType.mult,
                                           mybir.AluOpType.add)
            nc.vector.tensor_mul(d1[:], d1[:], a4_t[:])
            nc.scalar.activation(res_t[:], d1[:], AF.Sigmoid, scale=4.0)
            nc.sync.dma_start(out_d.ap(), res_t[:])

    nc.compile()
    return nc


# ---------------------------------------------------------------- runner

def _fingerprint(inputs):
    h = hashlib.md5()
    for k in sorted(inputs):
        a = np.asarray(inputs[k])
        h.update(k.encode())
        h.update(str(a.shape).encode())
        h.update(str(a.dtype).encode())
        flat = a.reshape(-1)
        stride = max(1, flat.size // 65536)
        h.update(np.ascontiguousarray(flat[::stride]).tobytes())
    return h.digest()


def _install_ntff_hook():
    """Provide antenv.axon_hooks (NTFF profiling over the axon tunnel) when
    the image lacks it: drives libaxon_pjrt.so's profile ABI via ctypes,
    mirroring trn_boot._ntff_profile_via_ctypes."""
    import types
    import ctypes
    import contextlib
    try:
        from antenv.axon_hooks import get_axon_ntff_profile_hook  # noqa: F401
        return True
    except ImportError:
        pass
    so_path = "/opt/axon/libaxon_pjrt.so"
    try:
        lib = ctypes.CDLL(so_path)
    except OSError:
        return False
    if not hasattr(lib, "axon_start_nrt_profile"):
        return False
    lib.axon_start_nrt_profile.argtypes = [ctypes.POINTER(ctypes.c_int64),
                                           ctypes.c_size_t]
    lib.axon_start_nrt_profile.restype = ctypes.c_int64
    lib.axon_stop_nrt_profile.argtypes = [ctypes.c_char_p]
    lib.axon_stop_nrt_profile.restype = ctypes.c_int64

    @contextlib.contextmanager
    def _hook(output_dir, device_ids):
        import jax
        jax.devices()
        if device_ids:
            ids = (ctypes.c_int64 * len(device_ids))(*device_ids)
            rc = lib.axon_start_nrt_profile(ids, len(device_ids))
        else:
            rc = lib.axon_start_nrt_profile(None, 0)
        if rc != 0:
            raise RuntimeError(f"axon_start_nrt_profile rc={rc}")
        try:
            yield
        finally:
            n = lib.axon_stop_nrt_profile(str(output_dir).encode())
            if n < 0:
                raise RuntimeError(f"axon_stop_nrt_profile rc={n}")

    mod = types.ModuleType("antenv.axon_hooks")
    mod.get_axon_ntff_profile_hook = lambda: _hook
    mod.set_axon_ntff_profile_hook = lambda h: None
    import antenv
    sys.modules["antenv.axon_hooks"] = mod
    antenv.axon_hooks = mod
    return True


def profile(trace=True, trace_cores=None):
    """Run the cached program with NTFF tracing; returns BassKernelResults
    (exec_time_ns = on-device NEFF execution time). Call kernel() first."""
    import concourse.bass_utils as bu
    assert "nc" in _state, "call kernel() first to build/caches the program"
    _install_ntff_hook()
    bu.upload_artifacts = lambda d: "local"   # no artifact bucket here
    return bu.run_bass_kernel_spmd(_state["nc"], _state["in_maps"],
                                   core_ids=list(range(N_CORES)), trace=trace,
                                   trace_cores=trace_cores)


def kernel(**inputs):
    from concourse.bass_utils import run_bass_kernel_spmd

    fp = _fingerprint(inputs)
    cached = _state.get("fp")
    if cached != fp:
        in_maps, meta = _host_prep(inputs)
        key = (meta["P"], tuple(meta["nlo"]), tuple(meta["nhi"]))
        if _state.get("prog_key") != key:
            _state["nc"] = _build_program(meta["P"], meta["nlo"], meta["nhi"])
            _state["prog_key"] = key
        _state["in_maps"] = in_maps
        _state["meta"] = meta
        _state["fp"] = fp

    meta = _state["meta"]
    res = run_bass_kernel_spmd(_state["nc"], _state["in_maps"],
                               core_ids=list(range(N_CORES)))
    out = np.zeros((B, 1), np.float32)
    students = meta["students"]
    for c in range(N_CORES):
        r = res.results[c]["out"].reshape(-1)
        sel = students[c]
        valid = sel >= 0
        out[sel[valid], 0] = r[:len(sel)][valid]
    return out


# revision 50
# speedup vs baseline: 1.0129x; 1.0129x over previous
"""DeepIRT forward as a Bass/Tile kernel on 8 Trainium2 NeuronCores.

Sharding: pure data parallelism over students (B=4096 -> 8 cores).
Students are globally sorted by qid_len (descending) and dealt to cores so
that every core has an IDENTICAL length profile (dummy students pad the
profile); this lets one SPMD program use a compile-time ragged schedule for
the LSTM (step t only touches the first n_t sorted columns).

Per-core program layout (P students, P % 16 == 0):
  - students indexed g in [0,P); duo D = g//4 holds 4 students (a = g%4)
  - attention (per "batch" of 4 duos = 16 students):
      qid+stu rows DMA'd -> PE transpose -> qidT [64d, 51] per student
      kemb rows via dma_gather from knE -> cast bf16 (stage-2 lhsT)
                                        -> PE transpose -> kembT (stage-1 lhsT)
      stage1: scoresT[k,t] (+ mastery preact col) via 4 quadrant matmuls/duo
              + one bias-row matmul adding -1e9 to invalid k rows
      softmax: exp (masked by bias), denominators via ones-matmul,
               reciprocal, broadcast-back via matmul
      stage2: [bvecT | mastvec | avec] via 4 quadrant matmuls/duo
  - theta/a DNNs: shared-weight matmuls over all students at once
  - LSTM: 50 steps, students split lo(even g)/hi(odd g) column groups,
          ragged active prefix per step, gates on PSUM, ACT sigmoids/tanh
  - head: b = 4*tanh((h@L_Wo+bo)/2), p = sigmoid(4 * a4 * (theta - b4))

Outputs [1, P] per core are gathered and inverse-permuted on the host.
"""

import sys
import hashlib

import numpy as np
import ml_dtypes

for _p in ("/opt/trn_rl_repo",):
    if _p not in sys.path:
        sys.path.insert(0, _p)

B, T, K, D, H, HL, S, KN = 4096, 50, 32, 64, 256, 128, 100000, 1000
N_CORES = 8
NEG = -1.0e9

_state = {}

BF16 = ml_dtypes.bfloat16


# ---------------------------------------------------------------- host prep

def _host_prep(inputs):
    """Sort/deal/pad students; build per-core input arrays + schedule."""
    lens = np.asarray(inputs["qid_len"]).astype(np.int64)          # [B]
    counts = np.bincount(lens, minlength=T + 1)                    # index 0..50
    m = -(-counts // N_CORES)                                      # ceil
    m[0] = 0
    P0 = int(m[1:].sum())
    P = ((P0 + 15) // 16) * 16
    m[1] += P - P0

    # per-core identical length profile, descending
    profile = np.repeat(np.arange(T, 0, -1), m[T:0:-1])            # [P]
    assert profile.shape[0] == P

    students = -np.ones((N_CORES, P), np.int64)
    ptr = 0
    for l in range(T, 0, -1):
        idxs = np.where(lens == l)[0]
        for c in range(N_CORES):
            take = idxs[c::N_CORES]
            assert take.shape[0] <= m[l]
            students[c, ptr:ptr + take.shape[0]] = take
        ptr += m[l]
    assert ptr == P

    n_t = np.array([(profile > t).sum() for t in range(T)], np.int64)
    nlo = [int(x) for x in (n_t + 1) // 2]
    nhi = [int(x) for x in n_t // 2]

    ND, NB = P // 4, P // 16

    qidemb = np.asarray(inputs["qidemb"], np.float32)
    stuE = np.asarray(inputs["stuE"], np.float32)
    uididx = np.asarray(inputs["uididx"])
    kcodeidx = np.asarray(inputs["kcodeidx"])
    kcode_len = np.asarray(inputs["kcode_len"]).astype(np.int64)

    # weights (replicated)
    wts = {
        "wi": np.asarray(inputs["L_Wi"], np.float32).astype(BF16),          # [64,512]
        "wh": np.asarray(inputs["L_Wh"], np.float32).astype(BF16),          # [128,512]
        "tw1": np.asarray(inputs["T_W1"], np.float32).astype(BF16),         # [64,256]
        "aw1": np.asarray(inputs["A_W1"], np.float32).astype(BF16),
        "tw2": np.asarray(inputs["T_W2"], np.float32)[:, 0].reshape(2, 128).T.copy().astype(BF16),
        "aw2": np.asarray(inputs["A_W2"], np.float32)[:, 0].reshape(2, 128).T.copy().astype(BF16),
        "lwo": np.asarray(inputs["L_Wo"], np.float32).reshape(128, 1).astype(BF16),
        "lb": np.asarray(inputs["L_b"], np.float32).reshape(4, 128).T.copy(),
        "tb1": np.asarray(inputs["T_b1"], np.float32).reshape(2, 128).T.copy(),
        "ab1": np.asarray(inputs["A_b1"], np.float32).reshape(2, 128).T.copy(),
        "scal": np.array([[float(np.asarray(inputs["T_b2"]).reshape(-1)[0]),
                           float(np.asarray(inputs["A_b2"]).reshape(-1)[0]),
                           0.5 * float(np.asarray(inputs["L_bo"]).reshape(-1)[0])]],
                         np.float32),
        "kne": np.asarray(inputs["knE"], np.float32),                       # [1000,64]
    }
    # constant pattern tiles
    sumpat = np.zeros((128, 4), BF16)
    for a in range(4):
        sumpat[32 * a:32 * (a + 1), a] = 1
    bc_ev = np.zeros((4, 128), BF16)
    bc_ev[0, 0:64] = 1
    bc_ev[1, 64:128] = 1
    bc_od = np.zeros((4, 128), BF16)
    bc_od[2, 0:64] = 1
    bc_od[3, 64:128] = 1
    blk4 = np.zeros((4, 204), BF16)
    for j in range(4):
        blk4[j, 51 * j:51 * (j + 1)] = 1
    consts = {"sumpat": sumpat, "bc_ev": bc_ev, "bc_od": bc_od, "blk4": blk4}

    in_maps = []
    for c in range(N_CORES):
        sel = students[c]
        safe = np.where(sel >= 0, sel, 0)

        q = qidemb[safe]                                           # [P,50,64]
        st = stuE[uididx[safe]]                                    # [P,64]
        qid_plus = np.concatenate([q, st[:, None, :]], axis=1).astype(BF16)

        kc = kcodeidx[safe].astype(np.int16)                       # [P,32]
        kidx = np.zeros((NB, 16, 32), np.int16)
        i = np.arange(512)
        kcb = kc.reshape(NB, 16, 32)
        kidx[:, i % 16, i // 16] = kcb[:, i // 32, i % 32]

        kl = kcode_len[safe].reshape(ND, 4)                        # [ND,4]
        kk = np.arange(K)
        kmf3 = (kk[None, None, :] < kl[:, :, None])                # [ND,4,32]
        kmf = kmf3.transpose(1, 2, 0).reshape(128, ND).astype(np.float32)
        # brow4[j, G, m] = bias of duo 4G+j at out1 row m (0 valid / -1e9 invalid)
        brow4 = np.where(kmf3, 0.0, NEG).reshape(NB, 4, 128).transpose(1, 0, 2) \
            .reshape(4, NB * 128).copy().astype(BF16)

        im = {"qid": qid_plus, "kidx": kidx, "kmf": kmf, "brow": brow4}
        im.update(wts)
        im.update(consts)
        in_maps.append(im)

    meta = {"P": P, "nlo": nlo, "nhi": nhi, "students": students}
    return in_maps, meta


# ---------------------------------------------------------------- program

def _build_program(P, nlo, nhi, phases=("attn", "dnn", "lstm", "head")):
    import os as _os
    if _os.environ.get("KPHASES"):
        phases = tuple(_os.environ["KPHASES"].split(","))
    ATT = int(_os.environ.get("KATT", "9"))
    ATT2 = int(_os.environ.get("KATT2", "9"))
    AMASK = int(_os.environ.get("KAMASK", "15"))
    import concourse.bacc as bacc
    import concourse.bass as bass
    import concourse.tile as tile
    from concourse import mybir
    from concourse.masks import make_identity
    from concourse.library_config import mlp
    from concourse.tile import add_dep_helper as add_dep
    from contextlib import ExitStack

    dt = mybir.dt
    AF = mybir.ActivationFunctionType
    ND, NB, NP2 = P // 4, P // 16, P // 2

    nc = bacc.Bacc("TRN2", target_bir_lowering=False, debug=False,
                   enable_asserts=False)

    qid = nc.dram_tensor("qid", [P, 51, 64], dt.bfloat16, kind="ExternalInput")
    kidx = nc.dram_tensor("kidx", [NB, 16, 32], dt.int16, kind="ExternalInput")
    kmf_d = nc.dram_tensor("kmf", [128, ND], dt.float32, kind="ExternalInput")
    brow_d = nc.dram_tensor("brow", [4, NB * 128], dt.bfloat16, kind="ExternalInput")
    kne = nc.dram_tensor("kne", [KN, D], dt.float32, kind="ExternalInput")
    wi_d = nc.dram_tensor("wi", [64, 512], dt.bfloat16, kind="ExternalInput")
    wh_d = nc.dram_tensor("wh", [128, 512], dt.bfloat16, kind="ExternalInput")
    tw1_d = nc.dram_tensor("tw1", [64, 256], dt.bfloat16, kind="ExternalInput")
    aw1_d = nc.dram_tensor("aw1", [64, 256], dt.bfloat16, kind="ExternalInput")
    tw2_d = nc.dram_tensor("tw2", [128, 2], dt.bfloat16, kind="ExternalInput")
    aw2_d = nc.dram_tensor("aw2", [128, 2], dt.bfloat16, kind="ExternalInput")
    lwo_d = nc.dram_tensor("lwo", [128, 1], dt.bfloat16, kind="ExternalInput")
    lb_d = nc.dram_tensor("lb", [128, 4], dt.float32, kind="ExternalInput")
    tb1_d = nc.dram_tensor("tb1", [128, 2], dt.float32, kind="ExternalInput")
    ab1_d = nc.dram_tensor("ab1", [128, 2], dt.float32, kind="ExternalInput")
    scal_d = nc.dram_tensor("scal", [1, 3], dt.float32, kind="ExternalInput")
    sumpat_d = nc.dram_tensor("sumpat", [128, 4], dt.bfloat16, kind="ExternalInput")
    bcev_d = nc.dram_tensor("bc_ev", [4, 128], dt.bfloat16, kind="ExternalInput")
    bcod_d = nc.dram_tensor("bc_od", [4, 128], dt.bfloat16, kind="ExternalInput")
    blk4_d = nc.dram_tensor("blk4", [4, 204], dt.bfloat16, kind="ExternalInput")
    out_d = nc.dram_tensor("out", [1, P], dt.float32, kind="ExternalOutput")

    with tile.TileContext(nc) as tc, ExitStack() as ctx:
        const = ctx.enter_context(tc.tile_pool(name="const", bufs=1))
        state = ctx.enter_context(tc.tile_pool(name="state", bufs=1))

        nc.gpsimd.load_library(mlp)

        ident = const.tile([128, 128], dt.bfloat16)
        make_identity(nc, ident[:])

        def load(pool, shape, dty, dram, dma2=False):
            t = pool.tile(shape, dty, tag=f"c_{dram.name}", name=f"c_{dram.name}")
            if dma2:  # duplicate 64-row weight into both partition halves
                nc.sync.dma_start(t[0:64, :], dram.ap())
                nc.sync.dma_start(t[64:128, :], dram.ap())
            else:
                nc.sync.dma_start(t[:], dram.ap())
            return t

        kmf_t = load(const, [128, ND], dt.float32, kmf_d)
        brow_t = load(const, [4, NB * 128], dt.bfloat16, brow_d)
        wi_t = load(const, [128, 512], dt.bfloat16, wi_d, dma2=True)
        wh_t = load(const, [128, 512], dt.bfloat16, wh_d)
        tw1_t = load(const, [128, 256], dt.bfloat16, tw1_d, dma2=True)
        aw1_t = load(const, [128, 256], dt.bfloat16, aw1_d, dma2=True)
        tw2_t = load(const, [128, 2], dt.bfloat16, tw2_d)
        aw2_t = load(const, [128, 2], dt.bfloat16, aw2_d)
        lwo_t = load(const, [128, 1], dt.bfloat16, lwo_d)
        lb_t = load(const, [128, 4], dt.float32, lb_d)
        tb1_t = load(const, [128, 2], dt.float32, tb1_d)
        ab1_t = load(const, [128, 2], dt.float32, ab1_d)
        scal_t = load(const, [1, 3], dt.float32, scal_d)
        sumpat_t = load(const, [128, 4], dt.bfloat16, sumpat_d)
        bcev_t = load(const, [4, 128], dt.bfloat16, bcev_d)
        bcod_t = load(const, [4, 128], dt.bfloat16, bcod_d)
        blk4_t = load(const, [4, 204], dt.bfloat16, blk4_d)

        # gather index tile: [16, n/16] block replicated into all 8
        # partition groups (each GPSIMD Q7 core reads its own group)
        kidx_t = const.tile([128, NB * 32], dt.int16)
        for rep in range(8):
            nc.sync.dma_start(
                kidx_t[16 * rep:16 * (rep + 1), :].rearrange(
                    "p (n k) -> p n k", n=NB),
                kidx.ap().rearrange("n p k -> p n k"))

        # persistent tensors
        bvec = state.tile([128, T * NP2], dt.bfloat16)   # [d(half), t*NP2+col]
        mastav = state.tile([128, P], dt.bfloat16)       # [d(half), 2*slot+c]
        h_t = state.tile([128, P], dt.bfloat16)
        c_t = state.tile([128, P], dt.float32)
        theta_t = state.tile([1, P], dt.float32)
        a4_t = state.tile([1, P], dt.float32)
        b4_t = state.tile([1, P], dt.float32)
        res_t = state.tile([1, P], dt.float32)
        nc.vector.memset(h_t[:], 0.0)
        nc.vector.memset(c_t[:], 0.0)

        # ---------------- attention ----------------
        with ExitStack() as atx:
          if "attn" in phases:
            qin = atx.enter_context(tc.tile_pool(name="qin", bufs=3))
            sbA = atx.enter_context(tc.tile_pool(name="sbA", bufs=3))
            ps_q = atx.enter_context(tc.tile_pool(name="ps_q", bufs=1, space="PSUM"))
            ps_k = atx.enter_context(tc.tile_pool(name="ps_k", bufs=1, space="PSUM"))
            ps_1 = atx.enter_context(tc.tile_pool(name="ps_1", bufs=2, space="PSUM"))
            ps_2e = atx.enter_context(tc.tile_pool(name="ps_2e", bufs=1, space="PSUM"))
            ps_2o = atx.enter_context(tc.tile_pool(name="ps_2o", bufs=1, space="PSUM"))
            ps_d = atx.enter_context(tc.tile_pool(name="ps_d", bufs=1, space="PSUM"))
            ps_b = atx.enter_context(tc.tile_pool(name="ps_b", bufs=1, space="PSUM"))

            for G in range(NB):
                # qid rows for 16 students -> [102, 512] (8 pairs wide)
                qtile = qin.tile([102, 512], dt.bfloat16, tag="qtile")
                for s in range(16 if ATT >= 1 else 0):
                    nc.sync.dma_start(
                        qtile[51 * (s % 2):51 * (s % 2) + 51,
                              64 * (s // 2):64 * (s // 2) + 64],
                        qid.ap()[16 * G + s])

                # kemb gather: 512 rows of knE -> [128, 4, 64] f32
                gath = sbA.tile([128, 256], dt.float32, tag="gath")
                if ATT < 2:
                    nc.vector.memset(gath[:], 0.0)
                if ATT >= 2:
                    nc.gpsimd.dma_gather(
                        gath[:].rearrange("p (b e) -> p b e", b=4),
                        kne.ap(), kidx_t[:, 32 * G:32 * (G + 1)], 512, 512, 64)
                kc16 = sbA.tile([128, 256], dt.bfloat16, tag="kc16")
                if ATT >= 3:
                    nc.vector.tensor_copy(kc16[:], gath[:])
                else:
                    nc.vector.memset(kc16[:], 0.0)
                if ATT < 4:
                    continue

                # transposes; first per PSUM bank is start=True (marks the
                # whole bank pending-zero), the rest overwrite pending bytes
                qT_ps = ps_q.tile([64, 1024], dt.bfloat16, tag="qtps")
                first = None
                for p8 in range(8):
                    mm = nc.tensor.matmul(qT_ps[:, 102 * p8:102 * (p8 + 1)],
                                          qtile[0:102, 64 * p8:64 * (p8 + 1)],
                                          ident[0:102, 0:102],
                                          is_transpose=True,
                                          start=(p8 == 0), stop=(p8 == 7),
                                          skip_group_check=True)
                    if first is None:
                        first = mm
                    else:
                        add_dep(mm.ins, first.ins, reason="bank first-writer")
                kT_ps = ps_k.tile([64, 1024], dt.bfloat16, tag="ktps")
                first = None
                for dd in range(4):
                    mm = nc.tensor.matmul(kT_ps[:, 128 * dd:128 * (dd + 1)],
                                          kc16[0:128, 64 * dd:64 * (dd + 1)],
                                          ident[0:128, 0:128],
                                          is_transpose=True,
                                          start=(dd == 0), stop=(dd == 3),
                                          skip_group_check=True)
                    if first is None:
                        first = mm
                    else:
                        add_dep(mm.ins, first.ins, reason="bank first-writer")
                qT = sbA.tile([64, 816], dt.bfloat16, tag="qT")
                nc.vector.tensor_copy(qT[:], qT_ps[:, 0:816])
                kT = sbA.tile([64, 512], dt.bfloat16, tag="kT")
                nc.vector.tensor_copy(kT[:], kT_ps[:, 0:512])

                # stage 1: out1[32a+k, 51*dd+t] = scoresT (+ mastery col 50)
                # first writer: bias matmul filling the whole bank with the
                # -1e9 invalid-k bias (start=True), then 16 quadrant matmuls
                # accumulate the actual scores.
                if ATT < 5:
                    continue
                out1 = ps_1.tile([128, 512], dt.float32, tag="out1")
                bmm = nc.tensor.matmul(
                    out1[:, 0:204], brow_t[:, 128 * G:128 * (G + 1)], blk4_t[:],
                    start=True, stop=False, skip_group_check=True)
                for dd in range(4):
                    for a in range(4):
                        pr = 2 * dd + a // 2
                        rhs = qT[:, 102 * pr + 51 * (a % 2):102 * pr + 51 * (a % 2) + 51]
                        mm = nc.tensor.matmul(
                            out1[32 * a:32 * (a + 1), 51 * dd:51 * (dd + 1)],
                            kT[:, 128 * dd + 32 * a:128 * dd + 32 * (a + 1)],
                            rhs, start=False, stop=(dd == 3 and a == 3),
                            tile_position=(0, 32 * a), skip_group_check=True)
                        add_dep(mm.ins, bmm.ins, reason="bias first-writer")

                # softmax pieces
                if ATT < 6:
                    continue
                o1v = out1[:, 0:204].rearrange("p (d c) -> p d c", d=4)
                expw = sbA.tile([128, 208], dt.bfloat16, tag="expw")
                ewv = expw[:].rearrange("p (d c) -> p d c", d=4)
                nc.scalar.activation(ewv[:, :, 0:50], o1v[:, :, 0:50],
                                     AF.Exp, scale=0.15)
                mast = sbA.tile([128, 4], dt.float32, tag="mast")
                nc.scalar.activation(mast[:], o1v[:, :, 50:51].rearrange("p a o -> p (a o)"),
                                     AF.Sigmoid, scale=0.2)
                nc.vector.tensor_mul(ewv[:, :, 50:51].rearrange("p a o -> p (a o)"),
                                     mast[:], kmf_t[:, 4 * G:4 * G + 4])
                nc.vector.tensor_copy(ewv[:, :, 51:52].rearrange("p a o -> p (a o)"),
                                      kmf_t[:, 4 * G:4 * G + 4])

                # denominators
                if ATT < 7:
                    continue
                dps = ps_d.tile([4, 512], dt.float32, tag="dps")
                nc.tensor.matmul(dps[:, 0:208], sumpat_t[:], expw[:],
                                 start=True, stop=True)
                rden = sbA.tile([4, 208], dt.bfloat16, tag="rden")
                nc.vector.memset(rden[:], 1.0)
                with nc.allow_low_precision(reason="bf16 softmax denominators"):
                    nc.vector.reciprocal(
                        rden[:].rearrange("p (d c) -> p d c", d=4)[:, :, 0:50],
                        dps[:, 0:208].rearrange("p (d c) -> p d c", d=4)[:, :, 0:50])
                bc = ps_b.tile([128, 512], dt.float32, tag="bc")
                mev = nc.tensor.matmul(bc[:, 0:208], bcev_t[:], rden[:],
                                       start=True, stop=False,
                                       skip_group_check=True)
                mod = nc.tensor.matmul(bc[:, 208:416], bcod_t[:], rden[:],
                                       start=False, stop=True,
                                       skip_group_check=True)
                add_dep(mod.ins, mev.ins, reason="bank first-writer")

                # stage 2: [bvecT | mastvec | avec]; first writer per
                # partition half is start=True (duo 0, a=0/a=1)
                if ATT < 8:
                    continue
                # separate PSUM banks per slot parity: concurrent PE row-tiles
                # must not write the same bank+partition range
                out2e = ps_2e.tile([128, 512], dt.float32, tag="out2e")
                out2o = ps_2o.tile([128, 512], dt.float32, tag="out2o")
                out2_par = (out2e, out2o)
                firsts = [[None, None], [None, None]]   # [par][hh]
                for dd in range(4):
                    for a in range(4):
                        if not (AMASK >> a) & 1:
                            continue
                        par = a // 2
                        hh = a % 2
                        o2 = out2_par[par]
                        mm = nc.tensor.matmul(
                            o2[64 * hh:64 * hh + 64, 52 * dd:52 * (dd + 1)],
                            kc16[32 * a:32 * (a + 1), 64 * dd:64 * (dd + 1)],
                            expw[32 * a:32 * (a + 1), 52 * dd:52 * (dd + 1)],
                            start=(firsts[par][hh] is None), stop=True,
                            tile_position=(32 * a, 64 * hh),
                            skip_group_check=True)
                        if firsts[par][hh] is None:
                            firsts[par][hh] = mm
                        else:
                            add_dep(mm.ins, firsts[par][hh].ins,
                                    reason="bank first-writer")

                # normalize bvec and write into bvec tile
                # bvec col = t*NP2 + (8G + 2*dd + par), partition-half = g%2
                # (bc must come via SBUF: DVE can't read 2 PSUM operands)
                if ATT2 < 2:
                    continue
                bc_sb = sbA.tile([128, 416], dt.bfloat16, tag="bc_sb")
                nc.vector.tensor_copy(bc_sb[:], bc[:, 0:416])
                for half in range(2):
                    r = slice(64 * half, 64 * half + 64)
                    for par in range(2):
                        o2v = out2_par[par][r, 0:208].rearrange(
                            "p (d c) -> p d c", d=4)
                        src = o2v[:, :, 0:50]
                        bcv = bc_sb[r, 208 * par:208 * (par + 1)].rearrange(
                            "p (d c) -> p d c", d=4)[:, :, 0:50]
                        dstf = bvec[r].rearrange(
                            "p (t qh two) -> p qh two t", qh=NP2 // 2, two=2)
                        dd_dst = dstf[:, 4 * G:4 * G + 4, par, :]
                        nc.vector.tensor_mul(dd_dst, src, bcv)

                # mastvec/avec extraction
                if ATT2 < 3:
                    continue
                for par in range(2):
                    nc.vector.tensor_copy(
                        mastav[:, 16 * G:16 * (G + 1)].rearrange(
                            "p (blk q c) -> p blk q c", blk=4, q=2)[:, :, par, :],
                        out2_par[par][:, 0:208].rearrange(
                            "p (d c) -> p d c", d=4)[:, :, 50:52])

        # ---------------- theta / a DNNs ----------------
        with ExitStack() as dtx:
          if "dnn" in phases:
            sbD = dtx.enter_context(tc.tile_pool(name="sbD", bufs=2))
            ps_h = dtx.enter_context(tc.tile_pool(name="ps_h", bufs=2, space="PSUM"))
            ps_o = dtx.enter_context(tc.tile_pool(name="ps_o", bufs=2, space="PSUM"))

            mav = mastav[:].rearrange("p (s c) -> p s c", s=NP2)
            for net, (w1, b1, w2, sc) in enumerate(
                    [(tw1_t, tb1_t, tw2_t, 0), (aw1_t, ab1_t, aw2_t, 1)]):
                dstv = (theta_t if net == 0 else a4_t)[:].rearrange(
                    "o (q two) -> o q two", two=2)
                for half in range(2):
                    r = slice(64 * half, 64 * half + 64)
                    rhs = mav[r, :, net]                       # [64, NP2]
                    ops = ps_o.tile([1, 512], dt.float32, tag="ops")
                    omm0 = None
                    for b in range(2):
                        hps = ps_h.tile([128, 512], dt.float32, tag="hps")
                        nc.tensor.matmul(hps[:, 0:NP2],
                                         w1[r, 128 * b:128 * (b + 1)], rhs,
                                         start=True, stop=True,
                                         tile_position=(64 * half, 0))
                        t1b = sbD.tile([128, NP2], dt.bfloat16, tag="t1b")
                        nc.scalar.activation(t1b[:], hps[:, 0:NP2], AF.Tanh,
                                             bias=b1[:, b:b + 1])
                        omm = nc.tensor.matmul(ops[:, 0:NP2], w2[:, b:b + 1], t1b[:],
                                               start=(b == 0), stop=(b == 1),
                                               skip_group_check=True)
                        if b == 0:
                            omm0 = omm
                        else:
                            add_dep(omm.ins, omm0.ins,
                                    reason="accum first-writer")
                    nc.scalar.activation(dstv[:, :, half], ops[:, 0:NP2],
                                         AF.Identity, bias=scal_t[:, sc:sc + 1])
            # a4 = tanh(|a_pre| / 2)
            nc.scalar.activation(a4_t[:], a4_t[:], AF.Abs)
            nc.scalar.activation(a4_t[:], a4_t[:], AF.Tanh, scale=0.5)

        # ---------------- LSTM ----------------
        with ExitStack() as ltx:
          if "lstm" in phases:
            ps_g = ltx.enter_context(tc.tile_pool(name="ps_g", bufs=1, space="PSUM"))
            sbL = ltx.enter_context(tc.tile_pool(name="sbL", bufs=2))
            gps = {}
            for grp in (0, 1):
                for b in range(4):
                    gtile = ps_g.tile([128, 512], dt.float32,
                                      tag=f"g{grp}{b}", name=f"g{grp}{b}")
                    gps[(grp, b)] = gtile
            for t in range(T):
                for grp, n in ((0, nlo[t]), (1, nhi[t])):
                    if n == 0:
                        continue
                    cb = NP2 * grp
                    wr = 64 * grp
                    # Wi first: it has no dependency on h, so the PE can
                    # pre-accumulate step t's input projection while the
                    # previous step's gate math is still in flight; the
                    # h-dependent Wh matmul lands second (accumulate).
                    for b in range(4):
                        gp = gps[(grp, b)]
                        mmi = nc.tensor.matmul(gp[:, 0:n],
                                               wi_t[wr:wr + 64, 128 * b:128 * (b + 1)],
                                               bvec[wr:wr + 64, t * NP2:t * NP2 + n],
                                               start=True, stop=False,
                                               tile_position=(wr, 0),
                                               skip_group_check=True)
                        mmh = nc.tensor.matmul(gp[:, 0:n],
                                               wh_t[:, 128 * b:128 * (b + 1)],
                                               h_t[:, cb:cb + n],
                                               start=False, stop=True,
                                               skip_group_check=True)
                        add_dep(mmh.ins, mmi.ins, reason="accum first-writer")
                    si = sbL.tile([128, NP2], dt.float32, tag="si")
                    sf = sbL.tile([128, NP2], dt.float32, tag="sf")
                    tg = sbL.tile([128, NP2], dt.float32, tag="tg")
                    so = sbL.tile([128, NP2], dt.float32, tag="so")
                    nc.scalar.activation(si[:, 0:n], gps[(grp, 0)][:, 0:n],
                                         AF.Sigmoid, bias=lb_t[:, 0:1])
                    nc.scalar.activation(sf[:, 0:n], gps[(grp, 1)][:, 0:n],
                                         AF.Sigmoid, bias=lb_t[:, 1:2])
                    nc.scalar.activation(tg[:, 0:n], gps[(grp, 2)][:, 0:n],
                                         AF.Tanh, bias=lb_t[:, 2:3])
                    nc.scalar.activation(so[:, 0:n], gps[(grp, 3)][:, 0:n],
                                         AF.Sigmoid, bias=lb_t[:, 3:4])
                    t1 = sbL.tile([128, NP2], dt.float32, tag="t1")
                    nc.vector.tensor_mul(t1[:, 0:n], si[:, 0:n], tg[:, 0:n])
                    nc.vector.tensor_mul(c_t[:, cb:cb + n], c_t[:, cb:cb + n],
                                         sf[:, 0:n])
                    nc.vector.tensor_add(c_t[:, cb:cb + n], c_t[:, cb:cb + n],
                                         t1[:, 0:n])
                    tc2 = sbL.tile([128, NP2], dt.float32, tag="tc2")
                    nc.scalar.activation(tc2[:, 0:n], c_t[:, cb:cb + n], AF.Tanh)
                    nc.vector.tensor_mul(h_t[:, cb:cb + n], so[:, 0:n],
                                         tc2[:, 0:n])

        # ---------------- head + combine ----------------
        with ExitStack() as htx:
          if "head" in phases:
            ps_r = htx.enter_context(tc.tile_pool(name="ps_r", bufs=2, space="PSUM"))
            sbH = htx.enter_context(tc.tile_pool(name="sbH", bufs=2))
            b4v = b4_t[:].rearrange("o (q two) -> o q two", two=2)
            for half in range(2):
                bps = ps_r.tile([1, 512], dt.float32, tag="bps")
                nc.tensor.matmul(bps[:, 0:NP2], lwo_t[:],
                                 h_t[:, NP2 * half:NP2 * (half + 1)],
                                 start=True, stop=True)
                nc.scalar.activation(b4v[:, :, half], bps[:, 0:NP2], AF.Tanh,
                                     scale=0.5, bias=scal_t[:, 2:3])
            d1 = sbH.tile([1, P], dt.float32, tag="d1")
            # p = sigmoid(a*(t-b)) with a = 4*a4, b = 4*b4
            #   = sigmoid(4 * a4 * (theta - 4*b4))
            nc.vector.scalar_tensor_tensor(d1[:], b4_t[:], -4.0, theta_t[:],
                                           mybir.AluOpType.mult,
                                           mybir.AluOpType.add)
            nc.vector.tensor_mul(d1[:], d1[:], a4_t[:])
            nc.scalar.activation(res_t[:], d1[:], AF.Sigmoid, scale=4.0)
            nc.sync.dma_start(out_d.ap(), res_t[:])

    nc.compile()
    return nc


# ---------------------------------------------------------------- runner

def _fingerprint(inputs):
    h = hashlib.md5()
    for k in sorted(inputs):
        a = np.asarray(inputs[k])
        h.update(k.encode())
        h.update(str(a.shape).encode())
        h.update(str(a.dtype).encode())
        flat = a.reshape(-1)
        stride = max(1, flat.size // 65536)
        h.update(np.ascontiguousarray(flat[::stride]).tobytes())
    return h.digest()


def _install_ntff_hook():
    """Provide antenv.axon_hooks (NTFF profiling over the axon tunnel) when
    the image lacks it: drives libaxon_pjrt.so's profile ABI via ctypes,
    mirroring trn_boot._ntff_profile_via_ctypes."""
    import types
    import ctypes
    import contextlib
    try:
        from antenv.axon_hooks import get_axon_ntff_profile_hook  # noqa: F401
        return True
    except ImportError:
        pass
    so_path = "/opt/axon/libaxon_pjrt.so"
    try:
        lib = ctypes.CDLL(so_path)
    except OSError:
        return False
    if not hasattr(lib, "axon_start_nrt_profile"):
        return False
    lib.axon_start_nrt_profile.argtypes = [ctypes.POINTER(ctypes.c_int64),
                                           ctypes.c_size_t]
    lib.axon_start_nrt_profile.restype = ctypes.c_int64
    lib.axon_stop_nrt_profile.argtypes = [ctypes.c_char_p]
    lib.axon_stop_nrt_profile.restype = ctypes.c_int64

    @contextlib.contextmanager
    def _hook(output_dir, device_ids):
        import jax
        jax.devices()
        if device_ids:
            ids = (ctypes.c_int64 * len(device_ids))(*device_ids)
            rc = lib.axon_start_nrt_profile(ids, len(device_ids))
        else:
            rc = lib.axon_start_nrt_profile(None, 0)
        if rc != 0:
            raise RuntimeError(f"axon_start_nrt_profile rc={rc}")
        try:
            yield
        finally:
            n = lib.axon_stop_nrt_profile(str(output_dir).encode())
            if n < 0:
                raise RuntimeError(f"axon_stop_nrt_profile rc={n}")

    mod = types.ModuleType("antenv.axon_hooks")
    mod.get_axon_ntff_profile_hook = lambda: _hook
    mod.set_axon_ntff_profile_hook = lambda h: None
    import antenv
    sys.modules["antenv.axon_hooks"] = mod
    antenv.axon_hooks = mod
    return True


def profile(trace=True, trace_cores=None):
    """Run the cached program with NTFF tracing; returns BassKernelResults
    (exec_time_ns = on-device NEFF execution time). Call kernel() first."""
    import concourse.bass_utils as bu
    assert "nc" in _state, "call kernel() first to build/caches the program"
    _install_ntff_hook()
    bu.upload_artifacts = lambda d: "local"   # no artifact bucket here
    return bu.run_bass_kernel_spmd(_state["nc"], _state["in_maps"],
                                   core_ids=list(range(N_CORES)), trace=trace,
                                   trace_cores=trace_cores)


def kernel(**inputs):
    from concourse.bass_utils import run_bass_kernel_spmd

    fp = _fingerprint(inputs)
    cached = _state.get("fp")
    if cached != fp:
        in_maps, meta = _host_prep(inputs)
        key = (meta["P"], tuple(meta["nlo"]), tuple(meta["nhi"]))
        if _state.get("prog_key") != key:
            _state["nc"] = _build_program(meta["P"], meta["nlo"], meta["nhi"])
            _state["prog_key"] = key
        _state["in_maps"] = in_maps
        _state["meta"] = meta
        _state["fp"] = fp

    meta = _state["meta"]
    res = run_bass_kernel_spmd(_state["nc"], _state["in_maps"],
                               core_ids=list(range(N_CORES)))
    out = np.zeros((B, 1), np.float32)
    students = meta["students"]
    for c in range(N_CORES):
        r = res.results[c]["out"].reshape(-1)
        sel = students[c]
        valid = sel >= 0
        out[sel[valid], 0] = r[:len(sel)][valid]
    return out


# revision 51
# speedup vs baseline: 1.0162x; 1.0032x over previous
"""DeepIRT forward as a Bass/Tile kernel on 8 Trainium2 NeuronCores.

Sharding: pure data parallelism over students (B=4096 -> 8 cores).
Students are globally sorted by qid_len (descending) and dealt to cores so
that every core has an IDENTICAL length profile (dummy students pad the
profile); this lets one SPMD program use a compile-time ragged schedule for
the LSTM (step t only touches the first n_t sorted columns).

Per-core program layout (P students, P % 16 == 0):
  - students indexed g in [0,P); duo D = g//4 holds 4 students (a = g%4)
  - attention (per "batch" of 4 duos = 16 students):
      qid+stu rows DMA'd -> PE transpose -> qidT [64d, 51] per student
      kemb rows via dma_gather from knE -> cast bf16 (stage-2 lhsT)
                                        -> PE transpose -> kembT (stage-1 lhsT)
      stage1: scoresT[k,t] (+ mastery preact col) via 4 quadrant matmuls/duo
              + one bias-row matmul adding -1e9 to invalid k rows
      softmax: exp (masked by bias), denominators via ones-matmul,
               reciprocal, broadcast-back via matmul
      stage2: [bvecT | mastvec | avec] via 4 quadrant matmuls/duo
  - theta/a DNNs: shared-weight matmuls over all students at once
  - LSTM: 50 steps, students split lo(even g)/hi(odd g) column groups,
          ragged active prefix per step, gates on PSUM, ACT sigmoids/tanh
  - head: b = 4*tanh((h@L_Wo+bo)/2), p = sigmoid(4 * a4 * (theta - b4))

Outputs [1, P] per core are gathered and inverse-permuted on the host.
"""

import sys
import hashlib

import numpy as np
import ml_dtypes

for _p in ("/opt/trn_rl_repo",):
    if _p not in sys.path:
        sys.path.insert(0, _p)

B, T, K, D, H, HL, S, KN = 4096, 50, 32, 64, 256, 128, 100000, 1000
N_CORES = 8
NEG = -1.0e9

_state = {}

BF16 = ml_dtypes.bfloat16


# ---------------------------------------------------------------- host prep

def _host_prep(inputs):
    """Sort/deal/pad students; build per-core input arrays + schedule."""
    lens = np.asarray(inputs["qid_len"]).astype(np.int64)          # [B]
    counts = np.bincount(lens, minlength=T + 1)                    # index 0..50
    m = -(-counts // N_CORES)                                      # ceil
    m[0] = 0
    P0 = int(m[1:].sum())
    P = ((P0 + 15) // 16) * 16
    m[1] += P - P0

    # per-core identical length profile, descending
    profile = np.repeat(np.arange(T, 0, -1), m[T:0:-1])            # [P]
    assert profile.shape[0] == P

    students = -np.ones((N_CORES, P), np.int64)
    ptr = 0
    for l in range(T, 0, -1):
        idxs = np.where(lens == l)[0]
        for c in range(N_CORES):
            take = idxs[c::N_CORES]
            assert take.shape[0] <= m[l]
            students[c, ptr:ptr + take.shape[0]] = take
        ptr += m[l]
    assert ptr == P

    n_t = np.array([(profile > t).sum() for t in range(T)], np.int64)
    nlo = [int(x) for x in (n_t + 1) // 2]
    nhi = [int(x) for x in n_t // 2]

    ND, NB = P // 4, P // 16

    qidemb = np.asarray(inputs["qidemb"], np.float32)
    stuE = np.asarray(inputs["stuE"], np.float32)
    uididx = np.asarray(inputs["uididx"])
    kcodeidx = np.asarray(inputs["kcodeidx"])
    kcode_len = np.asarray(inputs["kcode_len"]).astype(np.int64)

    # weights (replicated)
    wts = {
        "wi": np.asarray(inputs["L_Wi"], np.float32).astype(BF16),          # [64,512]
        "wh": np.asarray(inputs["L_Wh"], np.float32).astype(BF16),          # [128,512]
        "tw1": np.asarray(inputs["T_W1"], np.float32).astype(BF16),         # [64,256]
        "aw1": np.asarray(inputs["A_W1"], np.float32).astype(BF16),
        "tw2": np.asarray(inputs["T_W2"], np.float32)[:, 0].reshape(2, 128).T.copy().astype(BF16),
        "aw2": np.asarray(inputs["A_W2"], np.float32)[:, 0].reshape(2, 128).T.copy().astype(BF16),
        "lwo": np.asarray(inputs["L_Wo"], np.float32).reshape(128, 1).astype(BF16),
        "lb": np.asarray(inputs["L_b"], np.float32).reshape(4, 128).T.copy(),
        "tb1": np.asarray(inputs["T_b1"], np.float32).reshape(2, 128).T.copy(),
        "ab1": np.asarray(inputs["A_b1"], np.float32).reshape(2, 128).T.copy(),
        "scal": np.array([[float(np.asarray(inputs["T_b2"]).reshape(-1)[0]),
                           float(np.asarray(inputs["A_b2"]).reshape(-1)[0]),
                           0.5 * float(np.asarray(inputs["L_bo"]).reshape(-1)[0])]],
                         np.float32),
        "kne": np.asarray(inputs["knE"], np.float32),                       # [1000,64]
    }
    # constant pattern tiles
    sumpat = np.zeros((128, 4), BF16)
    for a in range(4):
        sumpat[32 * a:32 * (a + 1), a] = 1
    bc_ev = np.zeros((4, 128), BF16)
    bc_ev[0, 0:64] = 1
    bc_ev[1, 64:128] = 1
    bc_od = np.zeros((4, 128), BF16)
    bc_od[2, 0:64] = 1
    bc_od[3, 64:128] = 1
    blk4 = np.zeros((4, 204), BF16)
    for j in range(4):
        blk4[j, 51 * j:51 * (j + 1)] = 1
    consts = {"sumpat": sumpat, "bc_ev": bc_ev, "bc_od": bc_od, "blk4": blk4}

    in_maps = []
    for c in range(N_CORES):
        sel = students[c]
        safe = np.where(sel >= 0, sel, 0)

        q = qidemb[safe]                                           # [P,50,64]
        st = stuE[uididx[safe]]                                    # [P,64]
        qid_plus = np.concatenate([q, st[:, None, :]], axis=1).astype(BF16)

        kc = kcodeidx[safe].astype(np.int16)                       # [P,32]
        kidx = np.zeros((NB, 16, 32), np.int16)
        i = np.arange(512)
        kcb = kc.reshape(NB, 16, 32)
        kidx[:, i % 16, i // 16] = kcb[:, i // 32, i % 32]

        kl = kcode_len[safe].reshape(ND, 4)                        # [ND,4]
        kk = np.arange(K)
        kmf3 = (kk[None, None, :] < kl[:, :, None])                # [ND,4,32]
        kmf = kmf3.transpose(1, 2, 0).reshape(128, ND).astype(np.float32)
        # brow4[j, G, m] = bias of duo 4G+j at out1 row m (0 valid / -1e9 invalid)
        brow4 = np.where(kmf3, 0.0, NEG).reshape(NB, 4, 128).transpose(1, 0, 2) \
            .reshape(4, NB * 128).copy().astype(BF16)

        im = {"qid": qid_plus, "kidx": kidx, "kmf": kmf, "brow": brow4}
        im.update(wts)
        im.update(consts)
        in_maps.append(im)

    meta = {"P": P, "nlo": nlo, "nhi": nhi, "students": students}
    return in_maps, meta


# ---------------------------------------------------------------- program

def _build_program(P, nlo, nhi, phases=("attn", "dnn", "lstm", "head")):
    import os as _os
    if _os.environ.get("KPHASES"):
        phases = tuple(_os.environ["KPHASES"].split(","))
    ATT = int(_os.environ.get("KATT", "9"))
    ATT2 = int(_os.environ.get("KATT2", "9"))
    AMASK = int(_os.environ.get("KAMASK", "15"))
    import concourse.bacc as bacc
    import concourse.bass as bass
    import concourse.tile as tile
    from concourse import mybir
    from concourse.masks import make_identity
    from concourse.library_config import mlp
    from concourse.tile import add_dep_helper as add_dep
    from contextlib import ExitStack

    dt = mybir.dt
    AF = mybir.ActivationFunctionType
    ND, NB, NP2 = P // 4, P // 16, P // 2

    nc = bacc.Bacc("TRN2", target_bir_lowering=False, debug=False,
                   enable_asserts=False)

    qid = nc.dram_tensor("qid", [P, 51, 64], dt.bfloat16, kind="ExternalInput")
    kidx = nc.dram_tensor("kidx", [NB, 16, 32], dt.int16, kind="ExternalInput")
    kmf_d = nc.dram_tensor("kmf", [128, ND], dt.float32, kind="ExternalInput")
    brow_d = nc.dram_tensor("brow", [4, NB * 128], dt.bfloat16, kind="ExternalInput")
    kne = nc.dram_tensor("kne", [KN, D], dt.float32, kind="ExternalInput")
    wi_d = nc.dram_tensor("wi", [64, 512], dt.bfloat16, kind="ExternalInput")
    wh_d = nc.dram_tensor("wh", [128, 512], dt.bfloat16, kind="ExternalInput")
    tw1_d = nc.dram_tensor("tw1", [64, 256], dt.bfloat16, kind="ExternalInput")
    aw1_d = nc.dram_tensor("aw1", [64, 256], dt.bfloat16, kind="ExternalInput")
    tw2_d = nc.dram_tensor("tw2", [128, 2], dt.bfloat16, kind="ExternalInput")
    aw2_d = nc.dram_tensor("aw2", [128, 2], dt.bfloat16, kind="ExternalInput")
    lwo_d = nc.dram_tensor("lwo", [128, 1], dt.bfloat16, kind="ExternalInput")
    lb_d = nc.dram_tensor("lb", [128, 4], dt.float32, kind="ExternalInput")
    tb1_d = nc.dram_tensor("tb1", [128, 2], dt.float32, kind="ExternalInput")
    ab1_d = nc.dram_tensor("ab1", [128, 2], dt.float32, kind="ExternalInput")
    scal_d = nc.dram_tensor("scal", [1, 3], dt.float32, kind="ExternalInput")
    sumpat_d = nc.dram_tensor("sumpat", [128, 4], dt.bfloat16, kind="ExternalInput")
    bcev_d = nc.dram_tensor("bc_ev", [4, 128], dt.bfloat16, kind="ExternalInput")
    bcod_d = nc.dram_tensor("bc_od", [4, 128], dt.bfloat16, kind="ExternalInput")
    blk4_d = nc.dram_tensor("blk4", [4, 204], dt.bfloat16, kind="ExternalInput")
    out_d = nc.dram_tensor("out", [1, P], dt.float32, kind="ExternalOutput")

    with tile.TileContext(nc) as tc, ExitStack() as ctx:
        const = ctx.enter_context(tc.tile_pool(name="const", bufs=1))
        state = ctx.enter_context(tc.tile_pool(name="state", bufs=1))

        nc.gpsimd.load_library(mlp)

        ident = const.tile([128, 128], dt.bfloat16)
        make_identity(nc, ident[:])

        def load(pool, shape, dty, dram, dma2=False):
            t = pool.tile(shape, dty, tag=f"c_{dram.name}", name=f"c_{dram.name}")
            if dma2:  # duplicate 64-row weight into both partition halves
                nc.sync.dma_start(t[0:64, :], dram.ap())
                nc.sync.dma_start(t[64:128, :], dram.ap())
            else:
                nc.sync.dma_start(t[:], dram.ap())
            return t

        kmf_t = load(const, [128, ND], dt.float32, kmf_d)
        brow_t = load(const, [4, NB * 128], dt.bfloat16, brow_d)
        wi_t = load(const, [128, 512], dt.bfloat16, wi_d, dma2=True)
        wh_t = load(const, [128, 512], dt.bfloat16, wh_d)
        tw1_t = load(const, [128, 256], dt.bfloat16, tw1_d, dma2=True)
        aw1_t = load(const, [128, 256], dt.bfloat16, aw1_d, dma2=True)
        tw2_t = load(const, [128, 2], dt.bfloat16, tw2_d)
        aw2_t = load(const, [128, 2], dt.bfloat16, aw2_d)
        lwo_t = load(const, [128, 1], dt.bfloat16, lwo_d)
        lb_t = load(const, [128, 4], dt.float32, lb_d)
        tb1_t = load(const, [128, 2], dt.float32, tb1_d)
        ab1_t = load(const, [128, 2], dt.float32, ab1_d)
        scal_t = load(const, [1, 3], dt.float32, scal_d)
        sumpat_t = load(const, [128, 4], dt.bfloat16, sumpat_d)
        bcev_t = load(const, [4, 128], dt.bfloat16, bcev_d)
        bcod_t = load(const, [4, 128], dt.bfloat16, bcod_d)
        blk4_t = load(const, [4, 204], dt.bfloat16, blk4_d)

        # gather index tile: [16, n/16] block replicated into all 8
        # partition groups (each GPSIMD Q7 core reads its own group)
        kidx_t = const.tile([128, NB * 32], dt.int16)
        for rep in range(8):
            nc.sync.dma_start(
                kidx_t[16 * rep:16 * (rep + 1), :].rearrange(
                    "p (n k) -> p n k", n=NB),
                kidx.ap().rearrange("n p k -> p n k"))

        # persistent tensors
        bvec = state.tile([128, T * NP2], dt.bfloat16)   # [d(half), t*NP2+col]
        mastav = state.tile([128, P], dt.bfloat16)       # [d(half), 2*slot+c]
        h_t = state.tile([128, P], dt.bfloat16)
        c_t = state.tile([128, P], dt.float32)
        theta_t = state.tile([1, P], dt.float32)
        a4_t = state.tile([1, P], dt.float32)
        b4_t = state.tile([1, P], dt.float32)
        res_t = state.tile([1, P], dt.float32)
        nc.vector.memset(h_t[:], 0.0)
        nc.vector.memset(c_t[:], 0.0)

        # ---------------- attention ----------------
        with ExitStack() as atx:
          if "attn" in phases:
            qin = atx.enter_context(tc.tile_pool(name="qin", bufs=3))
            sbA = atx.enter_context(tc.tile_pool(name="sbA", bufs=3))
            ps_q = atx.enter_context(tc.tile_pool(name="ps_q", bufs=1, space="PSUM"))
            ps_k = atx.enter_context(tc.tile_pool(name="ps_k", bufs=1, space="PSUM"))
            ps_1 = atx.enter_context(tc.tile_pool(name="ps_1", bufs=1, space="PSUM"))
            ps_2e = atx.enter_context(tc.tile_pool(name="ps_2e", bufs=2, space="PSUM"))
            ps_2o = atx.enter_context(tc.tile_pool(name="ps_2o", bufs=2, space="PSUM"))
            ps_b = atx.enter_context(tc.tile_pool(name="ps_b", bufs=1, space="PSUM"))

            for G in range(NB):
                # qid rows for 16 students -> [102, 512] (8 pairs wide)
                qtile = qin.tile([102, 512], dt.bfloat16, tag="qtile")
                for s in range(16 if ATT >= 1 else 0):
                    nc.sync.dma_start(
                        qtile[51 * (s % 2):51 * (s % 2) + 51,
                              64 * (s // 2):64 * (s // 2) + 64],
                        qid.ap()[16 * G + s])

                # kemb gather: 512 rows of knE -> [128, 4, 64] f32
                gath = sbA.tile([128, 256], dt.float32, tag="gath")
                if ATT < 2:
                    nc.vector.memset(gath[:], 0.0)
                if ATT >= 2:
                    nc.gpsimd.dma_gather(
                        gath[:].rearrange("p (b e) -> p b e", b=4),
                        kne.ap(), kidx_t[:, 32 * G:32 * (G + 1)], 512, 512, 64)
                kc16 = sbA.tile([128, 256], dt.bfloat16, tag="kc16")
                if ATT >= 3:
                    nc.vector.tensor_copy(kc16[:], gath[:])
                else:
                    nc.vector.memset(kc16[:], 0.0)
                if ATT < 4:
                    continue

                # transposes; first per PSUM bank is start=True (marks the
                # whole bank pending-zero), the rest overwrite pending bytes
                qT_ps = ps_q.tile([64, 1024], dt.bfloat16, tag="qtps")
                first = None
                for p8 in range(8):
                    mm = nc.tensor.matmul(qT_ps[:, 102 * p8:102 * (p8 + 1)],
                                          qtile[0:102, 64 * p8:64 * (p8 + 1)],
                                          ident[0:102, 0:102],
                                          is_transpose=True,
                                          start=(p8 == 0), stop=(p8 == 7),
                                          skip_group_check=True)
                    if first is None:
                        first = mm
                    else:
                        add_dep(mm.ins, first.ins, reason="bank first-writer")
                kT_ps = ps_k.tile([64, 1024], dt.bfloat16, tag="ktps")
                first = None
                for dd in range(4):
                    mm = nc.tensor.matmul(kT_ps[:, 128 * dd:128 * (dd + 1)],
                                          kc16[0:128, 64 * dd:64 * (dd + 1)],
                                          ident[0:128, 0:128],
                                          is_transpose=True,
                                          start=(dd == 0), stop=(dd == 3),
                                          skip_group_check=True)
                    if first is None:
                        first = mm
                    else:
                        add_dep(mm.ins, first.ins, reason="bank first-writer")
                qT = sbA.tile([64, 816], dt.bfloat16, tag="qT")
                nc.scalar.copy(qT[:], qT_ps[:, 0:816])
                kT = sbA.tile([64, 512], dt.bfloat16, tag="kT")
                nc.scalar.copy(kT[:], kT_ps[:, 0:512])

                # stage 1: out1[32a+k, 51*dd+t] = scoresT (+ mastery col 50)
                # first writer: bias matmul filling the whole bank with the
                # -1e9 invalid-k bias (start=True), then 16 quadrant matmuls
                # accumulate the actual scores.
                if ATT < 5:
                    continue
                out1 = ps_1.tile([128, 512], dt.float32, tag="out1")
                bmm = nc.tensor.matmul(
                    out1[:, 0:204], brow_t[:, 128 * G:128 * (G + 1)], blk4_t[:],
                    start=True, stop=False, skip_group_check=True)
                for dd in range(4):
                    for a in range(4):
                        pr = 2 * dd + a // 2
                        rhs = qT[:, 102 * pr + 51 * (a % 2):102 * pr + 51 * (a % 2) + 51]
                        mm = nc.tensor.matmul(
                            out1[32 * a:32 * (a + 1), 51 * dd:51 * (dd + 1)],
                            kT[:, 128 * dd + 32 * a:128 * dd + 32 * (a + 1)],
                            rhs, start=False, stop=(dd == 3 and a == 3),
                            tile_position=(0, 32 * a), skip_group_check=True)
                        add_dep(mm.ins, bmm.ins, reason="bias first-writer")

                # softmax pieces
                if ATT < 6:
                    continue
                o1v = out1[:, 0:204].rearrange("p (d c) -> p d c", d=4)
                expw = sbA.tile([128, 208], dt.bfloat16, tag="expw")
                ewv = expw[:].rearrange("p (d c) -> p d c", d=4)
                nc.scalar.activation(ewv[:, :, 0:50], o1v[:, :, 0:50],
                                     AF.Exp, scale=0.15)
                mast = sbA.tile([128, 4], dt.float32, tag="mast")
                nc.scalar.activation(mast[:], o1v[:, :, 50:51].rearrange("p a o -> p (a o)"),
                                     AF.Sigmoid, scale=0.2)
                nc.vector.tensor_mul(ewv[:, :, 50:51].rearrange("p a o -> p (a o)"),
                                     mast[:], kmf_t[:, 4 * G:4 * G + 4])
                nc.vector.tensor_copy(ewv[:, :, 51:52].rearrange("p a o -> p (a o)"),
                                      kmf_t[:, 4 * G:4 * G + 4])

                # denominators
                if ATT < 7:
                    continue
                # denominators accumulate into spare columns of the out1
                # bank (rows 0-3); out1 data already consumed by exp/sigmoid
                dps = out1[0:4, 208:416]
                nc.tensor.matmul(dps, sumpat_t[:], expw[:],
                                 start=True, stop=True,
                                 skip_group_check=True)
                rden = sbA.tile([4, 208], dt.bfloat16, tag="rden")
                nc.vector.memset(rden[:], 1.0)
                with nc.allow_low_precision(reason="bf16 softmax denominators"):
                    nc.vector.reciprocal(
                        rden[:].rearrange("p (d c) -> p d c", d=4)[:, :, 0:50],
                        dps.rearrange("p (d c) -> p d c", d=4)[:, :, 0:50])
                bc = ps_b.tile([128, 512], dt.float32, tag="bc")
                mev = nc.tensor.matmul(bc[:, 0:208], bcev_t[:], rden[:],
                                       start=True, stop=False,
                                       skip_group_check=True)
                mod = nc.tensor.matmul(bc[:, 208:416], bcod_t[:], rden[:],
                                       start=False, stop=True,
                                       skip_group_check=True)
                add_dep(mod.ins, mev.ins, reason="bank first-writer")

                # stage 2: [bvecT | mastvec | avec]; first writer per
                # partition half is start=True (duo 0, a=0/a=1)
                if ATT < 8:
                    continue
                # separate PSUM banks per slot parity: concurrent PE row-tiles
                # must not write the same bank+partition range
                out2e = ps_2e.tile([128, 512], dt.float32, tag="out2e")
                out2o = ps_2o.tile([128, 512], dt.float32, tag="out2o")
                out2_par = (out2e, out2o)
                firsts = [[None, None], [None, None]]   # [par][hh]
                for dd in range(4):
                    for a in range(4):
                        if not (AMASK >> a) & 1:
                            continue
                        par = a // 2
                        hh = a % 2
                        o2 = out2_par[par]
                        mm = nc.tensor.matmul(
                            o2[64 * hh:64 * hh + 64, 52 * dd:52 * (dd + 1)],
                            kc16[32 * a:32 * (a + 1), 64 * dd:64 * (dd + 1)],
                            expw[32 * a:32 * (a + 1), 52 * dd:52 * (dd + 1)],
                            start=(firsts[par][hh] is None), stop=True,
                            tile_position=(32 * a, 64 * hh),
                            skip_group_check=True)
                        if firsts[par][hh] is None:
                            firsts[par][hh] = mm
                        else:
                            add_dep(mm.ins, firsts[par][hh].ins,
                                    reason="bank first-writer")

                # normalize bvec and write into bvec tile
                # bvec col = t*NP2 + (8G + 2*dd + par), partition-half = g%2
                # (bc must come via SBUF: DVE can't read 2 PSUM operands)
                if ATT2 < 2:
                    continue
                bc_sb = sbA.tile([128, 416], dt.bfloat16, tag="bc_sb")
                nc.vector.tensor_copy(bc_sb[:], bc[:, 0:416])
                for half in range(2):
                    r = slice(64 * half, 64 * half + 64)
                    for par in range(2):
                        o2v = out2_par[par][r, 0:208].rearrange(
                            "p (d c) -> p d c", d=4)
                        src = o2v[:, :, 0:50]
                        bcv = bc_sb[r, 208 * par:208 * (par + 1)].rearrange(
                            "p (d c) -> p d c", d=4)[:, :, 0:50]
                        dstf = bvec[r].rearrange(
                            "p (t qh two) -> p qh two t", qh=NP2 // 2, two=2)
                        dd_dst = dstf[:, 4 * G:4 * G + 4, par, :]
                        nc.vector.tensor_mul(dd_dst, src, bcv)

                # mastvec/avec extraction
                if ATT2 < 3:
                    continue
                for par in range(2):
                    nc.vector.tensor_copy(
                        mastav[:, 16 * G:16 * (G + 1)].rearrange(
                            "p (blk q c) -> p blk q c", blk=4, q=2)[:, :, par, :],
                        out2_par[par][:, 0:208].rearrange(
                            "p (d c) -> p d c", d=4)[:, :, 50:52])

        # ---------------- theta / a DNNs ----------------
        with ExitStack() as dtx:
          if "dnn" in phases:
            sbD = dtx.enter_context(tc.tile_pool(name="sbD", bufs=2))
            ps_h = dtx.enter_context(tc.tile_pool(name="ps_h", bufs=2, space="PSUM"))
            ps_o = dtx.enter_context(tc.tile_pool(name="ps_o", bufs=2, space="PSUM"))

            mav = mastav[:].rearrange("p (s c) -> p s c", s=NP2)
            for net, (w1, b1, w2, sc) in enumerate(
                    [(tw1_t, tb1_t, tw2_t, 0), (aw1_t, ab1_t, aw2_t, 1)]):
                dstv = (theta_t if net == 0 else a4_t)[:].rearrange(
                    "o (q two) -> o q two", two=2)
                for half in range(2):
                    r = slice(64 * half, 64 * half + 64)
                    rhs = mav[r, :, net]                       # [64, NP2]
                    ops = ps_o.tile([1, 512], dt.float32, tag="ops")
                    omm0 = None
                    for b in range(2):
                        hps = ps_h.tile([128, 512], dt.float32, tag="hps")
                        nc.tensor.matmul(hps[:, 0:NP2],
                                         w1[r, 128 * b:128 * (b + 1)], rhs,
                                         start=True, stop=True,
                                         tile_position=(64 * half, 0))
                        t1b = sbD.tile([128, NP2], dt.bfloat16, tag="t1b")
                        nc.scalar.activation(t1b[:], hps[:, 0:NP2], AF.Tanh,
                                             bias=b1[:, b:b + 1])
                        omm = nc.tensor.matmul(ops[:, 0:NP2], w2[:, b:b + 1], t1b[:],
                                               start=(b == 0), stop=(b == 1),
                                               skip_group_check=True)
                        if b == 0:
                            omm0 = omm
                        else:
                            add_dep(omm.ins, omm0.ins,
                                    reason="accum first-writer")
                    nc.scalar.activation(dstv[:, :, half], ops[:, 0:NP2],
                                         AF.Identity, bias=scal_t[:, sc:sc + 1])
            # a4 = tanh(|a_pre| / 2)
            nc.scalar.activation(a4_t[:], a4_t[:], AF.Abs)
            nc.scalar.activation(a4_t[:], a4_t[:], AF.Tanh, scale=0.5)

        # ---------------- LSTM ----------------
        with ExitStack() as ltx:
          if "lstm" in phases:
            ps_g = ltx.enter_context(tc.tile_pool(name="ps_g", bufs=1, space="PSUM"))
            sbL = ltx.enter_context(tc.tile_pool(name="sbL", bufs=2))
            gps = {}
            for grp in (0, 1):
                for b in range(4):
                    gtile = ps_g.tile([128, 512], dt.float32,
                                      tag=f"g{grp}{b}", name=f"g{grp}{b}")
                    gps[(grp, b)] = gtile
            for t in range(T):
                for grp, n in ((0, nlo[t]), (1, nhi[t])):
                    if n == 0:
                        continue
                    cb = NP2 * grp
                    wr = 64 * grp
                    # Wi first: it has no dependency on h, so the PE can
                    # pre-accumulate step t's input projection while the
                    # previous step's gate math is still in flight; the
                    # h-dependent Wh matmul lands second (accumulate).
                    for b in range(4):
                        gp = gps[(grp, b)]
                        mmi = nc.tensor.matmul(gp[:, 0:n],
                                               wi_t[wr:wr + 64, 128 * b:128 * (b + 1)],
                                               bvec[wr:wr + 64, t * NP2:t * NP2 + n],
                                               start=True, stop=False,
                                               tile_position=(wr, 0),
                                               skip_group_check=True)
                        mmh = nc.tensor.matmul(gp[:, 0:n],
                                               wh_t[:, 128 * b:128 * (b + 1)],
                                               h_t[:, cb:cb + n],
                                               start=False, stop=True,
                                               skip_group_check=True)
                        add_dep(mmh.ins, mmi.ins, reason="accum first-writer")
                    si = sbL.tile([128, NP2], dt.float32, tag="si")
                    sf = sbL.tile([128, NP2], dt.float32, tag="sf")
                    tg = sbL.tile([128, NP2], dt.float32, tag="tg")
                    so = sbL.tile([128, NP2], dt.float32, tag="so")
                    nc.scalar.activation(si[:, 0:n], gps[(grp, 0)][:, 0:n],
                                         AF.Sigmoid, bias=lb_t[:, 0:1])
                    nc.scalar.activation(sf[:, 0:n], gps[(grp, 1)][:, 0:n],
                                         AF.Sigmoid, bias=lb_t[:, 1:2])
                    nc.scalar.activation(tg[:, 0:n], gps[(grp, 2)][:, 0:n],
                                         AF.Tanh, bias=lb_t[:, 2:3])
                    nc.scalar.activation(so[:, 0:n], gps[(grp, 3)][:, 0:n],
                                         AF.Sigmoid, bias=lb_t[:, 3:4])
                    t1 = sbL.tile([128, NP2], dt.float32, tag="t1")
                    nc.vector.tensor_mul(t1[:, 0:n], si[:, 0:n], tg[:, 0:n])
                    nc.vector.tensor_mul(c_t[:, cb:cb + n], c_t[:, cb:cb + n],
                                         sf[:, 0:n])
                    nc.vector.tensor_add(c_t[:, cb:cb + n], c_t[:, cb:cb + n],
                                         t1[:, 0:n])
                    tc2 = sbL.tile([128, NP2], dt.float32, tag="tc2")
                    nc.scalar.activation(tc2[:, 0:n], c_t[:, cb:cb + n], AF.Tanh)
                    nc.vector.tensor_mul(h_t[:, cb:cb + n], so[:, 0:n],
                                         tc2[:, 0:n])

        # ---------------- head + combine ----------------
        with ExitStack() as htx:
          if "head" in phases:
            ps_r = htx.enter_context(tc.tile_pool(name="ps_r", bufs=2, space="PSUM"))
            sbH = htx.enter_context(tc.tile_pool(name="sbH", bufs=2))
            b4v = b4_t[:].rearrange("o (q two) -> o q two", two=2)
            for half in range(2):
                bps = ps_r.tile([1, 512], dt.float32, tag="bps")
                nc.tensor.matmul(bps[:, 0:NP2], lwo_t[:],
                                 h_t[:, NP2 * half:NP2 * (half + 1)],
                                 start=True, stop=True)
                nc.scalar.activation(b4v[:, :, half], bps[:, 0:NP2], AF.Tanh,
                                     scale=0.5, bias=scal_t[:, 2:3])
            d1 = sbH.tile([1, P], dt.float32, tag="d1")
            # p = sigmoid(a*(t-b)) with a = 4*a4, b = 4*b4
            #   = sigmoid(4 * a4 * (theta - 4*b4))
            nc.vector.scalar_tensor_tensor(d1[:], b4_t[:], -4.0, theta_t[:],
                                           mybir.AluOpType.mult,
                                           mybir.AluOpType.add)
            nc.vector.tensor_mul(d1[:], d1[:], a4_t[:])
            nc.scalar.activation(res_t[:], d1[:], AF.Sigmoid, scale=4.0)
            nc.sync.dma_start(out_d.ap(), res_t[:])

    nc.compile()
    return nc


# ---------------------------------------------------------------- runner

def _fingerprint(inputs):
    h = hashlib.md5()
    for k in sorted(inputs):
        a = np.asarray(inputs[k])
        h.update(k.encode())
        h.update(str(a.shape).encode())
        h.update(str(a.dtype).encode())
        flat = a.reshape(-1)
        stride = max(1, flat.size // 65536)
        h.update(np.ascontiguousarray(flat[::stride]).tobytes())
    return h.digest()


def _install_ntff_hook():
    """Provide antenv.axon_hooks (NTFF profiling over the axon tunnel) when
    the image lacks it: drives libaxon_pjrt.so's profile ABI via ctypes,
    mirroring trn_boot._ntff_profile_via_ctypes."""
    import types
    import ctypes
    import contextlib
    try:
        from antenv.axon_hooks import get_axon_ntff_profile_hook  # noqa: F401
        return True
    except ImportError:
        pass
    so_path = "/opt/axon/libaxon_pjrt.so"
    try:
        lib = ctypes.CDLL(so_path)
    except OSError:
        return False
    if not hasattr(lib, "axon_start_nrt_profile"):
        return False
    lib.axon_start_nrt_profile.argtypes = [ctypes.POINTER(ctypes.c_int64),
                                           ctypes.c_size_t]
    lib.axon_start_nrt_profile.restype = ctypes.c_int64
    lib.axon_stop_nrt_profile.argtypes = [ctypes.c_char_p]
    lib.axon_stop_nrt_profile.restype = ctypes.c_int64

    @contextlib.contextmanager
    def _hook(output_dir, device_ids):
        import jax
        jax.devices()
        if device_ids:
            ids = (ctypes.c_int64 * len(device_ids))(*device_ids)
            rc = lib.axon_start_nrt_profile(ids, len(device_ids))
        else:
            rc = lib.axon_start_nrt_profile(None, 0)
        if rc != 0:
            raise RuntimeError(f"axon_start_nrt_profile rc={rc}")
        try:
            yield
        finally:
            n = lib.axon_stop_nrt_profile(str(output_dir).encode())
            if n < 0:
                raise RuntimeError(f"axon_stop_nrt_profile rc={n}")

    mod = types.ModuleType("antenv.axon_hooks")
    mod.get_axon_ntff_profile_hook = lambda: _hook
    mod.set_axon_ntff_profile_hook = lambda h: None
    import antenv
    sys.modules["antenv.axon_hooks"] = mod
    antenv.axon_hooks = mod
    return True


def profile(trace=True, trace_cores=None):
    """Run the cached program with NTFF tracing; returns BassKernelResults
    (exec_time_ns = on-device NEFF execution time). Call kernel() first."""
    import concourse.bass_utils as bu
    assert "nc" in _state, "call kernel() first to build/caches the program"
    _install_ntff_hook()
    bu.upload_artifacts = lambda d: "local"   # no artifact bucket here
    return bu.run_bass_kernel_spmd(_state["nc"], _state["in_maps"],
                                   core_ids=list(range(N_CORES)), trace=trace,
                                   trace_cores=trace_cores)


def kernel(**inputs):
    from concourse.bass_utils import run_bass_kernel_spmd

    fp = _fingerprint(inputs)
    cached = _state.get("fp")
    if cached != fp:
        in_maps, meta = _host_prep(inputs)
        key = (meta["P"], tuple(meta["nlo"]), tuple(meta["nhi"]))
        if _state.get("prog_key") != key:
            _state["nc"] = _build_program(meta["P"], meta["nlo"], meta["nhi"])
            _state["prog_key"] = key
        _state["in_maps"] = in_maps
        _state["meta"] = meta
        _state["fp"] = fp

    meta = _state["meta"]
    res = run_bass_kernel_spmd(_state["nc"], _state["in_maps"],
                               core_ids=list(range(N_CORES)))
    out = np.zeros((B, 1), np.float32)
    students = meta["students"]
    for c in range(N_CORES):
        r = res.results[c]["out"].reshape(-1)
        sel = students[c]
        valid = sel >= 0
        out[sel[valid], 0] = r[:len(sel)][valid]
    return out
